# revision 1
# baseline (speedup 1.0000x reference)
"""Multi-head attention (B=4, S=2048, E=1024, H=16, D=64) on 8 Trainium2 cores.

Sharding: core c handles batch b=c//2 and head-group g=c%2 (8 heads, 4 pairs).

Per-core schedule (single fused stream, ACT-engine bound):
  prologue: V = x @ w_v (+bias) for all heads; QK proj for pair 0.
  for pair u: for head, for s-half: pipelined scores (bf16 matmul) ->
    exp (scalar engine, fp8 out) -> P@V (fp8 DoubleRow matmul, ones column
    gives softmax denominators). QK proj of pair u+1 and output proj of
    pair u-1 are drained into the PE stream as filler quanta so the tensor
    engine never idles while the scalar engine works through the exps.
  Normalization per (head, s-half): reciprocal of denominators (DVE),
    partition-broadcast via DRAM bounce, multiply on gpsimd.
  Output proj per pair -> PSUM -> SBUF -> DRAM partial; host sums the
  4 pair-partials x 2 head-group cores per batch and adds b_out.

Dtypes: x/weights bf16, scores psum f32, exp out fp8e4m3 (values are
unnormalized, ~e^|s|<13, well inside fp8 range), V fp8, attn bf16,
output f32. Offline numerics: rel err ~1.3e-2 (gate 2e-2).
"""
import os
import sys

sys.path.insert(0, "/opt/trn_rl_repo")

import numpy as np
import ml_dtypes

import concourse.bass as bass
import concourse.mybir as mybir
import concourse.tile as tile
from concourse import bacc
from concourse.bass_utils import run_bass_kernel_spmd

B, S, E, H, D = 4, 2048, 1024, 16, 64
HPC = 8            # heads per core
NPAIR = 4
NCORES = 8
P = 128
NST = S // P       # 16 s-tiles of 128
NSH = 2            # s-halves of 1024 per head
NSKP = NST // 2    # skt pairs
f32 = mybir.dt.float32
bf16 = mybir.dt.bfloat16
fp8 = mybir.dt.float8e4
AF = mybir.ActivationFunctionType
DR = mybir.MatmulPerfMode.DoubleRow
SCALE = 1.0 / 8.0  # 1/sqrt(D)

import os as _os
MODE = _os.environ.get("KERNEL_MODE", "v3")  # v3 | v2 | probe512

_BUILD_CACHE = {}
LAST_RESULTS = None


def build_nc(repeat=1, mode=MODE, bodies=1):
    # mode "probe512": v3 but exp emitted as 2x 512-wide chunks (ACT
    # instruction-overhead probe for HW-vs-sim calibration)
    # mode "v4": v3 + fp8 DoubleRow scores (Q,K quantized to fp8)
    pv_dt = fp8 if mode in ("v3", "probe512", "v4") else bf16
    qk_dt = fp8 if mode == "v4" else bf16
    # DoubleRow ldweights wants 128-wide planes; v2 (no DR) packs tighter
    PW = P if pv_dt == fp8 else D + 4
    PT_BUFS = 8 if pv_dt == fp8 else 4
    nc = bacc.Bacc("TRN2", target_bir_lowering=False, debug=False,
                   num_devices=NCORES)

    xT = nc.dram_tensor("xT", [E, S], bf16, kind="ExternalInput").ap()
    w_qk = nc.dram_tensor("w_qk", [E, HPC * P], bf16, kind="ExternalInput").ap()
    b_qk = nc.dram_tensor("b_qk", [HPC * P, 1], f32, kind="ExternalInput").ap()
    w_v = nc.dram_tensor("w_v", [E, HPC * D], bf16, kind="ExternalInput").ap()
    b_v = nc.dram_tensor("b_v", [P, HPC * D], f32, kind="ExternalInput").ap()
    w_out = nc.dram_tensor("w_out", [HPC * D, E], bf16, kind="ExternalInput").ap()
    outT = nc.dram_tensor("outT", [NPAIR, E, S], bf16, kind="ExternalOutput").ap()

    xT_r = xT.rearrange("(ko p) s -> p ko s", p=P)          # [128, 8, S]
    wqk_r = w_qk.rearrange("(ko p) f -> p ko f", p=P)       # [128, 8, 1024]
    wv_r = w_v.rearrange("(ko p) f -> p ko f", p=P)         # [128, 8, 512]
    bqk_r = b_qk.rearrange("(m p) one -> p (m one)", p=P)   # [128, 8]
    bv_r = b_v.rearrange("p (h d) -> p h d", d=D)           # [128, 8, 64]
    wo_r = w_out.rearrange("(j p) f -> p j f", p=P)         # [128, 4, 1024]
    outT_r = outT.rearrange("u (m p) s -> p u m s", p=P)    # [128, 4, 8, S]

    with tile.TileContext(nc) as tc:
        def body():
            from contextlib import ExitStack
            with ExitStack() as outer:
                persist = outer.enter_context(tc.tile_pool(name="persist", bufs=1))
                xsb = persist.tile([P, 8, S], bf16)
                wqk_sb = persist.tile([P, 8, HPC * P], bf16)
                wv_sb = persist.tile([P, 8, HPC * D], bf16)
                wo_sb = persist.tile([P, NPAIR, E], bf16)
                bqk_sb = persist.tile([P, 8], f32)
                bv_sb = persist.tile([P, HPC, D], f32)
                qT2 = persist.tile([P, NPAIR, S], qk_dt)
                kT2 = persist.tile([P, NPAIR, S], qk_dt)
                # V with ones column (softmax denominators), zero-padded to
                # 128-wide planes: the dual-fp8 DoubleRow ldweights ISA check
                # wants full 128-column weight planes (as tile_matmul uses).
                # dims [part, skp, head, skt-parity, 128], planes adjacent
                v_sb = persist.tile([P, NSKP, HPC, 2, PW], pv_dt)

                attnT_pool = outer.enter_context(
                    tc.tile_pool(name="attnT", bufs=NPAIR))
                if mode == "v4":
                    # remapped fp8 Q/K for DoubleRow scores:
                    # [32 part (d_lo), parity, d_hi plane, s]
                    q8_pool = outer.enter_context(
                        tc.tile_pool(name="q8", bufs=2))
                    k8_pool = outer.enter_context(
                        tc.tile_pool(name="k8", bufs=2))
                pT_pool = outer.enter_context(tc.tile_pool(name="pT", bufs=PT_BUFS))
                rec_pool = outer.enter_context(tc.tile_pool(name="rec", bufs=2))
                bc_pool = outer.enter_context(tc.tile_pool(name="bc", bufs=2))
                osb_pool = outer.enter_context(tc.tile_pool(name="osb", bufs=6))
                psS = outer.enter_context(
                    tc.tile_pool(name="psS", bufs=2, space="PSUM"))
                psA = outer.enter_context(
                    tc.tile_pool(name="psA", bufs=1, space="PSUM"))
                psX = outer.enter_context(
                    tc.tile_pool(name="psX", bufs=2, space="PSUM"))

                # ---- input DMAs: biases + x chunk 0 + w_qk first (QK proj
                # of pair 0 gates everything); k-halves split for earlier
                # start
                nc.sync.dma_start(bqk_sb[:], bqk_r)
                nc.sync.dma_start(xsb[:, 0:4, 0:512], xT_r[:, 0:4, 0:512])
                nc.sync.dma_start(wqk_sb[:, 0:4, :], wqk_r[:, 0:4, :])
                nc.sync.dma_start(xsb[:, 4:8, 0:512], xT_r[:, 4:8, 0:512])
                nc.sync.dma_start(wqk_sb[:, 4:8, :], wqk_r[:, 4:8, :])
                nc.sync.dma_start(xsb[:, :, 512:1024], xT_r[:, :, 512:1024])
                # second DMA queue (scalar engine, idle in prologue) for the
                # non-critical loads
                nc.scalar.dma_start(wv_sb[:], wv_r)
                nc.scalar.dma_start(bv_sb[:], bv_r)
                for q in range(2, 4):
                    sq = slice(q * 512, (q + 1) * 512)
                    nc.scalar.dma_start(xsb[:, :, sq], xT_r[:, :, sq])
                nc.scalar.dma_start(wo_sb[:], wo_r)
                nc.gpsimd.memset(v_sb[:, :, :, :, D:D + 1], 1.0)
                nc.gpsimd.memset(v_sb[:, :, :, :, D + 1:PW], 0.0)
                # preload the Exp activation table off the critical path
                warm = rec_pool.tile([1, 1024], f32)
                nc.vector.memset(warm[0:1, 0:2], 0.0)
                nc.scalar.activation(warm[0:1, 0:2], warm[0:1, 0:2], AF.Exp)
                # warm the PE p-state during the DMA wait: ~3us of dummy
                # matmuls on the first-arrived tile (outputs never read)
                wps = psX.tile([8, 8], f32, name="pwarm", tag="psx")
                for _ in range(12):
                    nc.tensor.matmul(wps[:], lhsT=bqk_sb[:, 0:8],
                                     rhs=bqk_sb[:, 0:8], start=True, stop=True)

                # ---- B quantum: V projection for one (s-tile, head pair)
                def b_quant(st, pr):
                    ps = psX.tile([P, 2 * D], f32, name="psb", tag="psx")
                    for k in range(8):
                        nc.tensor.matmul(
                            ps[:],
                            lhsT=xsb[:, k, st * P:(st + 1) * P],
                            rhs=wv_sb[:, k, pr * 2 * D:(pr + 1) * 2 * D],
                            start=(k == 0), stop=(k == 7))
                    nc.vector.tensor_add(
                        v_sb[:, st // 2, 2 * pr:2 * pr + 2, st % 2, 0:D],
                        ps.rearrange("p (h d) -> p h d", d=D),
                        bv_sb[:, 2 * pr:2 * pr + 2, :])

                # ---- A quanta: QK projection for one (m-tile, q-chunk)
                open_psa = {}

                def a_quant(m, q, half):
                    sq = slice(q * 512, (q + 1) * 512)
                    if half == 0:
                        ps = psX.tile([P, 512], f32, name="psa", tag="psx")
                        open_psa[(m, q)] = ps
                    else:
                        ps = open_psa.pop((m, q))
                    for k in range(4 * half, 4 * half + 4):
                        nc.tensor.matmul(
                            ps[:], lhsT=wqk_sb[:, k, m * P:(m + 1) * P],
                            rhs=xsb[:, k, sq],
                            start=(k == 0), stop=(k == 7))
                    if half == 1:
                        dst = qT2 if m % 2 == 0 else kT2
                        nc.vector.tensor_scalar_add(
                            dst[:, m // 2, sq], ps[:], bqk_sb[:, m:m + 1])

                # Minimal inline prologue: first scores chunk needs K s-tiles
                # 0-3 (K q-chunk 0) and Q cols 0:1024 (Q q-chunks 0,1)
                inline_tags = set()
                for m, q in ((1, 0), (0, 0), (0, 1)):
                    a_quant(m, q, 0)
                    a_quant(m, q, 1)
                    inline_tags |= {("A", 0, m, q, 0), ("A", 0, m, q, 1)}

                # ---- filler queue: (tag, PE-cost-ns, closure)
                # tags: ("A", pair, m, q, half) / ("B", pair, st) / ("D",...)
                fillers = []
                emitted = set(inline_tags)

                def _run(f):
                    tag, _, fn = f
                    fn()
                    emitted.add(tag)

                pool_ns = [0.0]

                def drain_budget(ns):
                    pool_ns[0] = min(pool_ns[0] + ns, 1700.0)
                    while fillers and fillers[0][1] <= pool_ns[0]:
                        f = fillers.pop(0)
                        pool_ns[0] -= f[1]
                        _run(f)

                def need(tag):
                    while fillers and tag not in emitted:
                        _run(fillers.pop(0))

                A_COST, B_COST, D_COST = 852, 424, 213

                def af(m, q, h):
                    return (("A", m // 2, m, q, h), A_COST,
                            lambda: a_quant(m, q, h))

                def bf(st, pr):
                    return (("B", pr, st), B_COST, lambda: b_quant(st, pr))

                # deferred prologue, ordered so head 0's consumption paces:
                # V(st) needed by PV(skp=st//2), K q-chunk c by scores skt>=4c,
                # Q q-chunks 2,3 only by s-half 1
                def remap_k(u, k8t, c):
                    # kT2[d, u, 512c:512c+512] -> k8t[d%32... planes]
                    sc = slice(c * 512, (c + 1) * 512)
                    for par in range(2):
                        for dt in range(2):
                            base = 64 * par + 32 * dt
                            nc.sync.dma_start(
                                k8t[:, par, dt, sc],
                                kT2[base:base + 32, u, sc])

                def remap_q(u, q8t, sh):
                    sc = slice(sh * 1024, (sh + 1) * 1024)
                    for par in range(2):
                        for dt in range(2):
                            base = 64 * par + 32 * dt
                            nc.sync.dma_start(
                                q8t[:, par, dt, sc],
                                qT2[base:base + 32, u, sc])

                def rkf(u, k8t, c):
                    return (("RK", u, c), 100,
                            lambda: remap_k(u, k8t, c))

                def rqf(u, q8t, sh):
                    return (("RQ", u, sh), 100,
                            lambda: remap_q(u, q8t, sh))

                q8_tiles, k8_tiles = {}, {}
                if mode == "v4":
                    q8_tiles[0] = q8_pool.tile([32, 2, 2, S], fp8, name="q8t")
                    k8_tiles[0] = k8_pool.tile([32, 2, 2, S], fp8, name="k8t")

                def rk0(c):
                    return [rkf(0, k8_tiles[0], c)] if mode == "v4" else []

                def rq0(sh):
                    return [rqf(0, q8_tiles[0], sh)] if mode == "v4" else []

                front = []
                front += rk0(0) + rq0(0)
                front += [bf(st, 0) for st in (0, 1)]
                front += [af(1, 1, h) for h in (0, 1)]
                front += rk0(1)
                front += [bf(st, 0) for st in (2, 3)]
                front += [af(1, 2, h) for h in (0, 1)]
                front += rk0(2)
                front += [bf(st, 0) for st in (4, 5)]
                front += [af(1, 3, h) for h in (0, 1)]
                front += rk0(3)
                front += [bf(st, 0) for st in range(6, NST)]
                for q in (2, 3):
                    front += [af(0, q, h) for h in (0, 1)]
                front += rq0(1)
                fillers.extend(front)

                def d_quant(u, attnT_u, m, c, evac="dve", pool=None):
                    sc = slice(c * 512, (c + 1) * 512)
                    if pool is None:
                        ps = psX.tile([P, 512], f32, name="psd", tag="psx")
                    else:
                        ps = pool.tile([P, 1024], f32, name="ps_s",
                                       tag="ps_s")[:, 0:512]
                    nc.tensor.matmul(
                        ps[:], lhsT=wo_sb[:, u, m * P:(m + 1) * P],
                        rhs=attnT_u[:, sc], start=True, stop=True)
                    o = osb_pool.tile([P, 512], bf16)
                    if evac == "act":
                        nc.scalar.copy(o[:], ps[:])
                    else:
                        nc.vector.tensor_copy(o[:], ps[:])
                    nc.sync.dma_start(outT_r[:, u, m, sc], o[:])

                # ---- main loop over head pairs
                pending = [None]

                def flush_pending():
                    if pending[0] is not None:
                        pvfn, fin = pending[0]
                        pending[0] = None
                        pvfn()
                        if fin is not None:
                            fin()

                attnT_tiles = {}
                for u in range(NPAIR):
                    attnT_u = attnT_pool.tile([P, S], bf16)
                    attnT_tiles[u] = attnT_u

                    # prefetch fillers for pair u+1, ordered like the pair-0
                    # front so need() rarely has to force anything: K q-chunk
                    # c before the scores that read it, V s-tiles before
                    # their PV, Q chunks 2,3 last (s-half 1 only). D quanta
                    # for pair u-1 go behind (no deadline until the tail).
                    if u + 1 < NPAIR:
                        un = u + 1
                        mq, mk = 2 * un, 2 * un + 1
                        if mode == "v4":
                            q8_tiles[un] = q8_pool.tile([32, 2, 2, S], fp8, name="q8t")
                            k8_tiles[un] = k8_pool.tile([32, 2, 2, S], fp8, name="k8t")

                        def rkn(c, un=un):
                            return [rkf(un, k8_tiles[un], c)] \
                                if mode == "v4" else []

                        def rqn(sh, un=un):
                            return [rqf(un, q8_tiles[un], sh)] \
                                if mode == "v4" else []

                        pf = []
                        pf += [af(mk, 0, h) for h in (0, 1)]
                        pf += rkn(0)
                        pf += [af(mq, 0, h) for h in (0, 1)]
                        pf += [af(mq, 1, h) for h in (0, 1)]
                        pf += rqn(0)
                        pf += [bf(st, un) for st in (0, 1)]
                        pf += [af(mk, 1, h) for h in (0, 1)]
                        pf += rkn(1)
                        pf += [bf(st, un) for st in (2, 3)]
                        pf += [af(mk, 2, h) for h in (0, 1)]
                        pf += rkn(2)
                        pf += [bf(st, un) for st in (4, 5)]
                        pf += [af(mk, 3, h) for h in (0, 1)]
                        pf += rkn(3)
                        pf += [bf(st, un) for st in range(6, NST)]
                        pf += [af(mq, 2, h) for h in (0, 1)]
                        pf += [af(mq, 3, h) for h in (0, 1)]
                        pf += rqn(1)
                        fillers.extend(pf)
                    if u > 0:
                        up, at_p = u - 1, attnT_tiles[u - 1]
                        for m in range(8):
                            for c in range(4):
                                fillers.append(
                                    (("D", up, m, c), D_COST,
                                     (lambda m=m, c=c, up=up, at_p=at_p:
                                      d_quant(up, at_p, m, c))))

                    for par in range(2):
                        i = 2 * u + par
                        poff = 64 * par
                        QT = qT2[poff:poff + 64, u, :]
                        KT = kT2[poff:poff + 64, u, :]
                        for sh in range(NSH):
                            s0 = sh * 1024
                            need(("A", u, 2 * u, 2 * sh, 1))
                            need(("A", u, 2 * u, 2 * sh + 1, 1))
                            if mode == "v4":
                                need(("RQ", u, sh))
                            at = psA.tile([P, 1024], f32)

                            def mk_pv(skp, pt, at=at, i=i, u=u):
                                def pv():
                                    need(("B", u, 2 * skp + 1))
                                    if mode in ("v3", "probe512", "v4"):
                                        for c in range(2):
                                            sc = slice(c * 512, (c + 1) * 512)
                                            nc.tensor.matmul(
                                                at[:, sc],
                                                lhsT=v_sb[:, skp, i, :, :],
                                                rhs=pt[:, :, sc],
                                                start=(skp == 0),
                                                stop=(skp == NSKP - 1),
                                                perf_mode=DR)
                                    else:
                                        for sp in range(2):
                                            skt = 2 * skp + sp
                                            for c in range(2):
                                                sc = slice(c * 512,
                                                           (c + 1) * 512)
                                                nc.tensor.matmul(
                                                    at[0:D + 1, sc],
                                                    lhsT=v_sb[:, skp, i, sp,
                                                              0:D + 1],
                                                    rhs=pt[:, sp, sc],
                                                    start=(skt == 0),
                                                    stop=(skt == NST - 1))
                                return pv

                            def mk_fin(at=at, i=i, u=u, par=par, sh=sh,
                                       poff=poff, s0=s0, attnT_u=attnT_u):
                                def fin():
                                    # denominators -> reciprocal; evacuate
                                    # values; normalize via Pool broadcast
                                    rec = rec_pool.tile([1, 1024], f32)
                                    nc.vector.reciprocal(rec[:],
                                                         at[D:D + 1, :])
                                    nc.vector.tensor_copy(
                                        attnT_u[poff:poff + 64,
                                                s0:s0 + 1024],
                                        at[0:D, :])
                                    bc = bc_pool.tile([P, 1024], f32)
                                    nc.gpsimd.partition_broadcast(
                                        bc[:], rec[:], P)
                                    nc.gpsimd.tensor_mul(
                                        attnT_u[poff:poff + 64,
                                                s0:s0 + 1024],
                                        attnT_u[poff:poff + 64,
                                                s0:s0 + 1024],
                                        bc[poff:poff + 64, :])
                                    if u == NPAIR - 1 and par == 1:
                                        # last pair: tail D quanta, DVE/ACT
                                        # split, borrowing free score banks
                                        tailq = []
                                        for c in (2 * sh, 2 * sh + 1):
                                            for m in range(8):
                                                ev = "act" if (sh == 1 and
                                                               m % 2) \
                                                    else "dve"
                                                pl = psS if (
                                                    sh == 1 and
                                                    (m // 2) % 2) else None
                                                tailq.append(
                                                    (("D", u, m, c), D_COST,
                                                     (lambda m=m, c=c,
                                                      at_u=attnT_u, ev=ev,
                                                      pl=pl:
                                                      d_quant(NPAIR - 1,
                                                              at_u, m, c,
                                                              ev, pl))))
                                        fillers[:0] = tailq
                                return fin

                            for skp in range(NSKP):
                                pt = pT_pool.tile([P, 2, 1024], pv_dt)
                                for sp in range(2):
                                    skt = 2 * skp + sp
                                    need(("A", u, 2 * u + 1, skt // 4, 1))
                                    if mode == "v4":
                                        need(("RK", u, skt // 4))
                                    ps_s = psS.tile([P, 1024], f32,
                                                    name="ps_s", tag="ps_s")
                                    if mode == "v4":
                                        q8t = q8_tiles[u]
                                        k8t = k8_tiles[u]
                                        for c in range(2):
                                            nc.tensor.matmul(
                                                ps_s[:, c * 512:
                                                     (c + 1) * 512],
                                                lhsT=k8t[:, par, :,
                                                         skt * P:
                                                         (skt + 1) * P],
                                                rhs=q8t[:, par, :,
                                                        s0 + c * 512:
                                                        s0 + (c + 1) * 512],
                                                start=True, stop=True,
                                                perf_mode=DR)
                                    else:
                                        for c in range(2):
                                            nc.tensor.matmul(
                                                ps_s[:, c * 512:
                                                     (c + 1) * 512],
                                                lhsT=KT[:, skt * P:
                                                        (skt + 1) * P],
                                                rhs=QT[:, s0 + c * 512:
                                                       s0 + (c + 1) * 512],
                                                start=True, stop=True)
                                    if mode == "probe512":
                                        for ch in range(2):
                                            nc.scalar.activation(
                                                pt[:, sp, ch * 512:
                                                   (ch + 1) * 512],
                                                ps_s[:, ch * 512:
                                                     (ch + 1) * 512],
                                                AF.Exp, scale=SCALE)
                                    else:
                                        nc.scalar.activation(
                                            pt[:, sp, :], ps_s[:], AF.Exp,
                                            scale=SCALE)
                                # the previous chunk's PV (possibly from the
                                # previous s-half/head) flushes only after
                                # this chunk's scores+exp feed the ACT engine
                                flush_pending()
                                pending[0] = (
                                    mk_pv(skp, pt),
                                    mk_fin() if skp == NSKP - 1 else None)
                                drain_budget(1010)
                flush_pending()
                while fillers:
                    _run(fillers.pop(0))

        if repeat > 1:
            with tc.For_i(0, repeat, 1):
                for _ in range(bodies):
                    body()
        else:
            body()

    nc.compile()
    return nc


def _get_nc(repeat=1, mode=MODE, bodies=1):
    key = (repeat, mode, bodies)
    if key not in _BUILD_CACHE:
        _BUILD_CACHE[key] = build_nc(repeat=repeat, mode=mode, bodies=bodies)
    return _BUILD_CACHE[key]


def shard_inputs(x, w_qkv, b_qkv, w_out, b_out):
    """Host-side sharding: per-core input maps."""
    bf = ml_dtypes.bfloat16
    in_maps = []
    for c in range(NCORES):
        b, g = c // 2, c % 2
        heads = [g * HPC + i for i in range(HPC)]
        # qk columns, pair-interleaved: m-tile 2u = q cols of pair u,
        # m-tile 2u+1 = k cols of pair u; within a tile [headA 64 | headB 64]
        qk_cols, qk_bias = [], []
        for u in range(HPC // 2):
            hA, hB = heads[2 * u], heads[2 * u + 1]
            for off in (0, 64):  # 0: q, 64: k
                for h in (hA, hB):
                    qk_cols.append(w_qkv[:, h * 192 + off:h * 192 + off + 64])
                    qk_bias.append(b_qkv[h * 192 + off:h * 192 + off + 64])
        w_qk_c = np.ascontiguousarray(
            np.concatenate(qk_cols, axis=1)).astype(bf)
        b_qk_c = np.ascontiguousarray(
            np.concatenate(qk_bias)[:, None].astype(np.float32))
        w_v_c = np.ascontiguousarray(np.concatenate(
            [w_qkv[:, h * 192 + 128:h * 192 + 192] for h in heads],
            axis=1)).astype(bf)
        b_v_c = np.ascontiguousarray(np.broadcast_to(np.concatenate(
            [b_qkv[h * 192 + 128:h * 192 + 192] for h in heads])[None, :],
            (P, HPC * D)).astype(np.float32))
        w_out_c = np.ascontiguousarray(np.concatenate(
            [w_out[h * D:(h + 1) * D, :] for h in heads], axis=0)).astype(bf)
        xT_c = np.ascontiguousarray(x[b].T).astype(bf)
        in_maps.append({
            "xT": xT_c, "w_qk": w_qk_c, "b_qk": b_qk_c,
            "w_v": w_v_c, "b_v": b_v_c, "w_out": w_out_c,
        })
    return in_maps


def unshard_output(results, b_out):
    out = np.empty((B, S, E), dtype=np.float32)
    for b in range(B):
        acc = results[2 * b]["outT"].astype(np.float32).sum(axis=0)
        acc += results[2 * b + 1]["outT"].astype(np.float32).sum(axis=0)
        out[b] = acc.T + b_out
    return out


def kernel(x, w_qkv, b_qkv, w_out, b_out):
    global LAST_RESULTS
    x = np.asarray(x, dtype=np.float32)
    w_qkv = np.asarray(w_qkv, dtype=np.float32)
    b_qkv = np.asarray(b_qkv, dtype=np.float32)
    w_out = np.asarray(w_out, dtype=np.float32)
    b_out = np.asarray(b_out, dtype=np.float32)

    nc = _get_nc()
    in_maps = shard_inputs(x, w_qkv, b_qkv, w_out, b_out)
    try:
        res = run_bass_kernel_spmd(nc, in_maps, list(range(NCORES)))
    except ModuleNotFoundError:
        # BASS_TRACE requested but this axon client has no NTFF hook module
        os.environ["BASS_NEVER_TRACE"] = "1"
        res = run_bass_kernel_spmd(nc, in_maps, list(range(NCORES)))
    LAST_RESULTS = res
    return unshard_output(res.results, b_out)



# revision 4
# speedup vs baseline: 1.0899x; 1.0899x over previous
"""Multi-head attention (B=4, S=2048, E=1024, H=16, D=64) on 8 Trainium2 cores.

Sharding: core c handles batch b=c//2 and head-group g=c%2 (8 heads, 4 pairs).

Per-core schedule (single fused stream; ACT(exp)-bound on HW):
  prologue: V = x @ w_v (+bias) for all heads; QK proj for pair 0.
  for pair u: for head, for s-half: pipelined scores (bf16 matmul) ->
    exp (scalar engine, fp8 out) -> P@V (fp8 DoubleRow matmul, ones column
    gives softmax denominators). QK proj of pair u+1 and output proj of
    pair u-1 are drained into the PE stream as filler quanta so the tensor
    engine never idles while the scalar engine works through the exps.
  Normalization per (head, s-half): reciprocal of denominators (DVE),
    partition-broadcast (gpsimd), multiply on gpsimd.
  Output proj per pair -> PSUM -> SBUF -> DRAM partial; host sums the
  4 pair-partials x 2 head-group cores per batch and adds b_out.

HW calibration (measured via engine-pure probes, wall-clock-delta):
  PE matmul streams ~1 col/cycle @2.4GHz ONLY when lhsT occupies 128
  physical partitions; 64-partition weights run at HALF rate (and fp8
  DoubleRow with 64-partition 2-plane weights is half rate too — the rate
  follows physical partitions). Matmul out is capped at 512 elements.
  ACT exp [128,1024] f32(PSUM)->fp8: ~1.38us (0.9-1.0 ns/elem + ~290ns
  fixed + ~180ns PSUM read penalty). DVE copy [*,1024]: ~1.3us.

  Scores contract over D=64 only, so Q is stored ZERO-PADDED to 128
  partitions (per-head slot: valid 64 rows aligned with that head's rows
  in the packed K tile, zeros elsewhere) — the padded bf16 matmul runs at
  full rate, halving scores PE time with zero numerics change.

Dtypes: x/weights bf16, scores psum f32, exp out fp8e4m3, V fp8 (values
~e^|s| small), attn bf16, output f32. Offline numerics: rel err ~1.3e-2
(gate 2e-2).
"""
import os
import sys

sys.path.insert(0, "/opt/trn_rl_repo")

import numpy as np
import ml_dtypes

import concourse.bass as bass
import concourse.mybir as mybir
import concourse.tile as tile
from concourse import bacc
from concourse.bass_utils import run_bass_kernel_spmd

B, S, E, H, D = 4, 2048, 1024, 16, 64
HPC = 8            # heads per core
NPAIR = 4
NCORES = 8
P = 128
NST = S // P       # 16 s-tiles of 128
NSH = 2            # s-halves of 1024 per head
NSKP = NST // 2    # skt pairs
f32 = mybir.dt.float32
bf16 = mybir.dt.bfloat16
fp8 = mybir.dt.float8e4
AF = mybir.ActivationFunctionType
DR = mybir.MatmulPerfMode.DoubleRow
SCALE = 1.0 / 8.0  # 1/sqrt(D)

_BUILD_CACHE = {}
LAST_RESULTS = None

# HW-measured PE costs (ns) for the filler pacing heuristic
A_COST, B_COST, D_COST = 940, 660, 235
CHUNK_BUDGET = 1320    # ACT per-skp time (2x1.38us) minus pinned PE work
BUDGET_CAP = 1700


def build_nc(repeat=1, bodies=1):
    PW = P  # fp8 DoubleRow ldweights wants full 128-column weight planes
    nc = bacc.Bacc("TRN2", target_bir_lowering=False, debug=False,
                   num_devices=NCORES)

    xT = nc.dram_tensor("xT", [E, S], bf16, kind="ExternalInput").ap()
    w_qk = nc.dram_tensor("w_qk", [E, HPC * P], bf16, kind="ExternalInput").ap()
    b_qk = nc.dram_tensor("b_qk", [HPC * P, 1], f32, kind="ExternalInput").ap()
    w_v = nc.dram_tensor("w_v", [E, HPC * D], bf16, kind="ExternalInput").ap()
    b_v = nc.dram_tensor("b_v", [P, HPC * D], f32, kind="ExternalInput").ap()
    w_out = nc.dram_tensor("w_out", [HPC * D, E], bf16, kind="ExternalInput").ap()
    outT = nc.dram_tensor("outT", [NPAIR, E, S], bf16, kind="ExternalOutput").ap()

    xT_r = xT.rearrange("(ko p) s -> p ko s", p=P)          # [128, 8, S]
    wqk_r = w_qk.rearrange("(ko p) f -> p ko f", p=P)       # [128, 8, 1024]
    wv_r = w_v.rearrange("(ko p) f -> p ko f", p=P)         # [128, 8, 512]
    bqk_r = b_qk.rearrange("(m p) one -> p (m one)", p=P)   # [128, 8]
    bv_r = b_v.rearrange("p (h d) -> p h d", d=D)           # [128, 8, 64]
    wo_r = w_out.rearrange("(j p) f -> p j f", p=P)         # [128, 4, 1024]
    outT_r = outT.rearrange("u (m p) s -> p u m s", p=P)    # [128, 4, 8, S]

    with tile.TileContext(nc) as tc:
        def body():
            from contextlib import ExitStack
            with ExitStack() as outer:
                persist = outer.enter_context(tc.tile_pool(name="persist", bufs=1))
                xsb = persist.tile([P, 8, S], bf16)
                wqk_sb = persist.tile([P, 8, HPC * P], bf16)
                wv_sb = persist.tile([P, 8, HPC * D], bf16)
                wo_sb = persist.tile([P, NPAIR, E], bf16)
                bqk_sb = persist.tile([P, 8], f32)
                bv_sb = persist.tile([P, HPC, D], f32)
                # Q per-head zero-padded to 128 partitions: slot h holds head
                # h's q on the 64 partitions matching its rows in the packed
                # K tile (even h -> 0:64, odd h -> 64:128), zeros elsewhere,
                # so the contract-128 scores matmul streams at full rate.
                qPAD = persist.tile([P, HPC, S], bf16)
                kT2 = persist.tile([P, NPAIR, S], bf16)
                # V with ones column (softmax denominators), zero-padded to
                # 128-wide planes for the dual-fp8 DoubleRow ldweights check.
                # dims [part, skp, head, skt-parity, 128], planes adjacent
                v_sb = persist.tile([P, NSKP, HPC, 2, PW], fp8)

                attnT_pool = outer.enter_context(
                    tc.tile_pool(name="attnT", bufs=NPAIR))
                pT_pool = outer.enter_context(tc.tile_pool(name="pT", bufs=8))
                rec_pool = outer.enter_context(tc.tile_pool(name="rec", bufs=2))
                bc_pool = outer.enter_context(tc.tile_pool(name="bc", bufs=2))
                osb_pool = outer.enter_context(tc.tile_pool(name="osb", bufs=6))
                psS = outer.enter_context(
                    tc.tile_pool(name="psS", bufs=2, space="PSUM"))
                psA = outer.enter_context(
                    tc.tile_pool(name="psA", bufs=1, space="PSUM"))
                psX = outer.enter_context(
                    tc.tile_pool(name="psX", bufs=2, space="PSUM"))

                # ---- input DMAs: biases + x chunk 0 + w_qk first (QK proj
                # of pair 0 gates everything); k-halves split for earlier
                # start
                nc.sync.dma_start(bqk_sb[:], bqk_r)
                nc.sync.dma_start(xsb[:, 0:4, 0:512], xT_r[:, 0:4, 0:512])
                nc.sync.dma_start(wqk_sb[:, 0:4, :], wqk_r[:, 0:4, :])
                nc.sync.dma_start(xsb[:, 4:8, 0:512], xT_r[:, 4:8, 0:512])
                nc.sync.dma_start(wqk_sb[:, 4:8, :], wqk_r[:, 4:8, :])
                nc.sync.dma_start(xsb[:, :, 512:1024], xT_r[:, :, 512:1024])
                # second DMA queue (scalar engine, idle in prologue) for the
                # non-critical loads
                nc.scalar.dma_start(wv_sb[:], wv_r)
                nc.scalar.dma_start(bv_sb[:], bv_r)
                for q in range(2, 4):
                    sq = slice(q * 512, (q + 1) * 512)
                    nc.scalar.dma_start(xsb[:, :, sq], xT_r[:, :, sq])
                nc.scalar.dma_start(wo_sb[:], wo_r)
                nc.gpsimd.memset(v_sb[:, :, :, :, D:D + 1], 1.0)
                nc.gpsimd.memset(v_sb[:, :, :, :, D + 1:PW], 0.0)
                # zero the complementary halves of the padded-Q slots (never
                # written again; the zeros select one head in the packed K)
                for h in range(HPC):
                    if h % 2 == 0:
                        nc.gpsimd.memset(qPAD[64:128, h, :], 0.0)
                    else:
                        nc.gpsimd.memset(qPAD[0:64, h, :], 0.0)
                # preload the Exp activation table off the critical path
                warm = rec_pool.tile([1, 1024], f32)
                nc.vector.memset(warm[0:1, 0:2], 0.0)
                nc.scalar.activation(warm[0:1, 0:2], warm[0:1, 0:2], AF.Exp)
                # warm the PE p-state during the DMA wait: ~3us of dummy
                # matmuls on the first-arrived tile (outputs never read)
                wps = psX.tile([8, 8], f32, name="pwarm", tag="psx")
                for _ in range(12):
                    nc.tensor.matmul(wps[:], lhsT=bqk_sb[:, 0:8],
                                     rhs=bqk_sb[:, 0:8], start=True, stop=True)

                # ---- B quantum: V projection for one (s-tile, head pair)
                def b_quant(st, pr):
                    ps = psX.tile([P, 2 * D], f32, name="psb", tag="psx")
                    for k in range(8):
                        nc.tensor.matmul(
                            ps[:],
                            lhsT=xsb[:, k, st * P:(st + 1) * P],
                            rhs=wv_sb[:, k, pr * 2 * D:(pr + 1) * 2 * D],
                            start=(k == 0), stop=(k == 7))
                    nc.vector.tensor_add(
                        v_sb[:, st // 2, 2 * pr:2 * pr + 2, st % 2, 0:D],
                        ps.rearrange("p (h d) -> p h d", d=D),
                        bv_sb[:, 2 * pr:2 * pr + 2, :])

                # ---- A quanta: QK projection for one (m-tile, q-chunk)
                open_psa = {}

                def a_quant(m, q, half):
                    sq = slice(q * 512, (q + 1) * 512)
                    if half == 0:
                        ps = psX.tile([P, 512], f32, name="psa", tag="psx")
                        open_psa[(m, q)] = ps
                    else:
                        ps = open_psa.pop((m, q))
                    for k in range(4 * half, 4 * half + 4):
                        nc.tensor.matmul(
                            ps[:], lhsT=wqk_sb[:, k, m * P:(m + 1) * P],
                            rhs=xsb[:, k, sq],
                            start=(k == 0), stop=(k == 7))
                    if half == 1:
                        if m % 2 == 0:
                            # q of pair u=m//2: two half-partition writes into
                            # the per-head padded slots
                            u = m // 2
                            nc.vector.tensor_scalar_add(
                                qPAD[0:64, 2 * u, sq], ps[0:64, :],
                                bqk_sb[0:64, m:m + 1])
                            nc.vector.tensor_scalar_add(
                                qPAD[64:128, 2 * u + 1, sq], ps[64:128, :],
                                bqk_sb[64:128, m:m + 1])
                        else:
                            nc.vector.tensor_scalar_add(
                                kT2[:, m // 2, sq], ps[:], bqk_sb[:, m:m + 1])

                # Minimal inline prologue: first scores chunk needs K s-tiles
                # 0-3 (K q-chunk 0) and Q cols 0:1024 (Q q-chunks 0,1)
                inline_tags = set()
                for m, q in ((1, 0), (0, 0), (0, 1)):
                    a_quant(m, q, 0)
                    a_quant(m, q, 1)
                    inline_tags |= {("A", 0, m, q, 0), ("A", 0, m, q, 1)}

                # ---- filler queue: (tag, PE-cost-ns, closure)
                # tags: ("A", pair, m, q, half) / ("B", pair, st) / ("D",...)
                fillers = []
                emitted = set(inline_tags)

                def _run(f):
                    tag, _, fn = f
                    fn()
                    emitted.add(tag)

                pool_ns = [0.0]

                def drain_budget(ns):
                    pool_ns[0] = min(pool_ns[0] + ns, float(BUDGET_CAP))
                    while fillers and fillers[0][1] <= pool_ns[0]:
                        f = fillers.pop(0)
                        pool_ns[0] -= f[1]
                        _run(f)

                def need(tag):
                    while fillers and tag not in emitted:
                        _run(fillers.pop(0))

                def af(m, q, h):
                    return (("A", m // 2, m, q, h), A_COST,
                            lambda: a_quant(m, q, h))

                def bf(st, pr):
                    return (("B", pr, st), B_COST, lambda: b_quant(st, pr))

                # deferred prologue, ordered so head 0's consumption paces:
                # V(st) needed by PV(skp=st//2), K q-chunk c by scores skt>=4c,
                # Q q-chunks 2,3 only by s-half 1
                front = []
                front += [bf(st, 0) for st in (0, 1)]
                front += [af(1, 1, h) for h in (0, 1)]
                front += [bf(st, 0) for st in (2, 3)]
                front += [af(1, 2, h) for h in (0, 1)]
                front += [bf(st, 0) for st in (4, 5)]
                front += [af(1, 3, h) for h in (0, 1)]
                front += [bf(st, 0) for st in range(6, NST)]
                for q in (2, 3):
                    front += [af(0, q, h) for h in (0, 1)]
                fillers.extend(front)

                def d_quant(u, attnT_u, m, c, evac="dve", pool=None):
                    sc = slice(c * 512, (c + 1) * 512)
                    if pool is None:
                        ps = psX.tile([P, 512], f32, name="psd", tag="psx")
                    else:
                        ps = pool.tile([P, 1024], f32, name="ps_s",
                                       tag="ps_s")[:, 0:512]
                    nc.tensor.matmul(
                        ps[:], lhsT=wo_sb[:, u, m * P:(m + 1) * P],
                        rhs=attnT_u[:, sc], start=True, stop=True)
                    o = osb_pool.tile([P, 512], bf16)
                    if evac == "act":
                        nc.scalar.copy(o[:], ps[:])
                    else:
                        nc.vector.tensor_copy(o[:], ps[:])
                    nc.sync.dma_start(outT_r[:, u, m, sc], o[:])

                # ---- main loop over head pairs
                pending = [None]

                def flush_pending():
                    if pending[0] is not None:
                        pvfn, fin = pending[0]
                        pending[0] = None
                        pvfn()
                        if fin is not None:
                            fin()

                attnT_tiles = {}
                for u in range(NPAIR):
                    attnT_u = attnT_pool.tile([P, S], bf16)
                    attnT_tiles[u] = attnT_u

                    # prefetch fillers for pair u+1, ordered like the pair-0
                    # front so need() rarely has to force anything. D quanta
                    # for pair u-1 go behind (no deadline until the tail).
                    if u + 1 < NPAIR:
                        un = u + 1
                        mq, mk = 2 * un, 2 * un + 1
                        pf = []
                        pf += [af(mk, 0, h) for h in (0, 1)]
                        pf += [af(mq, 0, h) for h in (0, 1)]
                        pf += [af(mq, 1, h) for h in (0, 1)]
                        pf += [bf(st, un) for st in (0, 1)]
                        pf += [af(mk, 1, h) for h in (0, 1)]
                        pf += [bf(st, un) for st in (2, 3)]
                        pf += [af(mk, 2, h) for h in (0, 1)]
                        pf += [bf(st, un) for st in (4, 5)]
                        pf += [af(mk, 3, h) for h in (0, 1)]
                        pf += [bf(st, un) for st in range(6, NST)]
                        pf += [af(mq, 2, h) for h in (0, 1)]
                        pf += [af(mq, 3, h) for h in (0, 1)]
                        fillers.extend(pf)
                    if u > 0:
                        up, at_p = u - 1, attnT_tiles[u - 1]
                        for m in range(8):
                            for c in range(4):
                                fillers.append(
                                    (("D", up, m, c), D_COST,
                                     (lambda m=m, c=c, up=up, at_p=at_p:
                                      d_quant(up, at_p, m, c))))

                    for par in range(2):
                        i = 2 * u + par
                        poff = 64 * par
                        for sh in range(NSH):
                            s0 = sh * 1024
                            need(("A", u, 2 * u, 2 * sh, 1))
                            need(("A", u, 2 * u, 2 * sh + 1, 1))
                            at = psA.tile([P, 1024], f32)

                            def mk_pv(skp, pt, at=at, i=i, u=u):
                                def pv():
                                    need(("B", u, 2 * skp + 1))
                                    for c in range(2):
                                        sc = slice(c * 512, (c + 1) * 512)
                                        nc.tensor.matmul(
                                            at[:, sc],
                                            lhsT=v_sb[:, skp, i, :, :],
                                            rhs=pt[:, :, sc],
                                            start=(skp == 0),
                                            stop=(skp == NSKP - 1),
                                            perf_mode=DR)
                                return pv

                            def mk_fin(at=at, i=i, u=u, par=par, sh=sh,
                                       poff=poff, s0=s0, attnT_u=attnT_u):
                                def fin():
                                    # denominators -> reciprocal; evacuate
                                    # values; normalize via Pool broadcast
                                    rec = rec_pool.tile([1, 1024], f32)
                                    nc.vector.reciprocal(rec[:],
                                                         at[D:D + 1, :])
                                    nc.vector.tensor_copy(
                                        attnT_u[poff:poff + 64,
                                                s0:s0 + 1024],
                                        at[0:D, :])
                                    bc = bc_pool.tile([P, 1024], f32)
                                    nc.gpsimd.partition_broadcast(
                                        bc[:], rec[:], P)
                                    nc.gpsimd.tensor_mul(
                                        attnT_u[poff:poff + 64,
                                                s0:s0 + 1024],
                                        attnT_u[poff:poff + 64,
                                                s0:s0 + 1024],
                                        bc[poff:poff + 64, :])
                                    if u == NPAIR - 1 and par == 1:
                                        # last pair: tail D quanta, DVE/ACT
                                        # split, borrowing free score banks
                                        tailq = []
                                        for c in (2 * sh, 2 * sh + 1):
                                            for m in range(8):
                                                ev = "act" if (sh == 1 and
                                                               m % 2) \
                                                    else "dve"
                                                pl = psS if (
                                                    sh == 1 and
                                                    (m // 2) % 2) else None
                                                tailq.append(
                                                    (("D", u, m, c), D_COST,
                                                     (lambda m=m, c=c,
                                                      at_u=attnT_u, ev=ev,
                                                      pl=pl:
                                                      d_quant(NPAIR - 1,
                                                              at_u, m, c,
                                                              ev, pl))))
                                        fillers[:0] = tailq
                                return fin

                            for skp in range(NSKP):
                                pt = pT_pool.tile([P, 2, 1024], fp8)
                                for sp in range(2):
                                    skt = 2 * skp + sp
                                    need(("A", u, 2 * u + 1, skt // 4, 1))
                                    ps_s = psS.tile([P, 1024], f32,
                                                    name="ps_s", tag="ps_s")
                                    for c in range(2):
                                        # contract-128: packed K (both heads)
                                        # x zero-padded per-head Q slot
                                        nc.tensor.matmul(
                                            ps_s[:, c * 512:(c + 1) * 512],
                                            lhsT=kT2[:, u, skt * P:
                                                     (skt + 1) * P],
                                            rhs=qPAD[:, i,
                                                     s0 + c * 512:
                                                     s0 + (c + 1) * 512],
                                            start=True, stop=True)
                                    nc.scalar.activation(
                                        pt[:, sp, :], ps_s[:], AF.Exp,
                                        scale=SCALE)
                                # the previous chunk's PV (possibly from the
                                # previous s-half/head) flushes only after
                                # this chunk's scores+exp feed the ACT engine
                                flush_pending()
                                pending[0] = (
                                    mk_pv(skp, pt),
                                    mk_fin() if skp == NSKP - 1 else None)
                                drain_budget(CHUNK_BUDGET)
                flush_pending()
                while fillers:
                    _run(fillers.pop(0))

        if repeat > 1:
            with tc.For_i(0, repeat, 1):
                for _ in range(bodies):
                    body()
        else:
            body()

    nc.compile()
    return nc


def _get_nc(repeat=1, bodies=1):
    key = (repeat, bodies)
    if key not in _BUILD_CACHE:
        _BUILD_CACHE[key] = build_nc(repeat=repeat, bodies=bodies)
    return _BUILD_CACHE[key]


def shard_inputs(x, w_qkv, b_qkv, w_out, b_out):
    """Host-side sharding: per-core input maps."""
    bf = ml_dtypes.bfloat16
    in_maps = []
    for c in range(NCORES):
        b, g = c // 2, c % 2
        heads = [g * HPC + i for i in range(HPC)]
        # qk columns, pair-interleaved: m-tile 2u = q cols of pair u,
        # m-tile 2u+1 = k cols of pair u; within a tile [headA 64 | headB 64]
        qk_cols, qk_bias = [], []
        for u in range(HPC // 2):
            hA, hB = heads[2 * u], heads[2 * u + 1]
            for off in (0, 64):  # 0: q, 64: k
                for h in (hA, hB):
                    qk_cols.append(w_qkv[:, h * 192 + off:h * 192 + off + 64])
                    qk_bias.append(b_qkv[h * 192 + off:h * 192 + off + 64])
        w_qk_c = np.ascontiguousarray(
            np.concatenate(qk_cols, axis=1)).astype(bf)
        b_qk_c = np.ascontiguousarray(
            np.concatenate(qk_bias)[:, None].astype(np.float32))
        w_v_c = np.ascontiguousarray(np.concatenate(
            [w_qkv[:, h * 192 + 128:h * 192 + 192] for h in heads],
            axis=1)).astype(bf)
        b_v_c = np.ascontiguousarray(np.broadcast_to(np.concatenate(
            [b_qkv[h * 192 + 128:h * 192 + 192] for h in heads])[None, :],
            (P, HPC * D)).astype(np.float32))
        w_out_c = np.ascontiguousarray(np.concatenate(
            [w_out[h * D:(h + 1) * D, :] for h in heads], axis=0)).astype(bf)
        xT_c = np.ascontiguousarray(x[b].T).astype(bf)
        in_maps.append({
            "xT": xT_c, "w_qk": w_qk_c, "b_qk": b_qk_c,
            "w_v": w_v_c, "b_v": b_v_c, "w_out": w_out_c,
        })
    return in_maps


def unshard_output(results, b_out):
    out = np.empty((B, S, E), dtype=np.float32)
    for b in range(B):
        acc = results[2 * b]["outT"].astype(np.float32).sum(axis=0)
        acc += results[2 * b + 1]["outT"].astype(np.float32).sum(axis=0)
        out[b] = acc.T + b_out
    return out


def kernel(x, w_qkv, b_qkv, w_out, b_out):
    global LAST_RESULTS
    x = np.asarray(x, dtype=np.float32)
    w_qkv = np.asarray(w_qkv, dtype=np.float32)
    b_qkv = np.asarray(b_qkv, dtype=np.float32)
    w_out = np.asarray(w_out, dtype=np.float32)
    b_out = np.asarray(b_out, dtype=np.float32)

    nc = _get_nc()
    in_maps = shard_inputs(x, w_qkv, b_qkv, w_out, b_out)
    try:
        res = run_bass_kernel_spmd(nc, in_maps, list(range(NCORES)))
    except ModuleNotFoundError:
        # BASS_TRACE requested but this axon client has no NTFF hook module
        os.environ["BASS_NEVER_TRACE"] = "1"
        res = run_bass_kernel_spmd(nc, in_maps, list(range(NCORES)))
    LAST_RESULTS = res
    return unshard_output(res.results, b_out)


# revision 18
# speedup vs baseline: 1.2109x; 1.1111x over previous
"""Multi-head attention (B=4, S=2048, E=1024, H=16, D=64) on 8 Trainium2 cores.

Sharding: core c handles batch b=c//2 and head-group g=c%2 (8 heads, 4 pairs).

Per-core schedule (single fused stream; ACT(exp)-bound on HW):
  prologue: V = x @ w_v (+bias) for all heads; QK proj for pair 0.
  for pair u: for head, for s-half: pipelined scores (bf16 matmul) ->
    exp (scalar engine, fp8 out) -> P@V (fp8 DoubleRow matmul, ones column
    gives softmax denominators). QK proj of pair u+1 and output proj of
    pair u-1 are drained into the PE stream as filler quanta so the tensor
    engine never idles while the scalar engine works through the exps.
  Normalization per (head, s-half): reciprocal of denominators (DVE),
    partition-broadcast (gpsimd), multiply on gpsimd.
  Output proj per pair -> PSUM -> SBUF -> DRAM partial; host sums the
  4 pair-partials x 2 head-group cores per batch and adds b_out.

HW calibration (measured via engine-pure probes, wall-clock-delta):
  PE matmul streams ~1 col/cycle @2.4GHz ONLY when lhsT occupies 128
  physical partitions; 64-partition weights run at HALF rate (and fp8
  DoubleRow with 64-partition 2-plane weights is half rate too — the rate
  follows physical partitions). Matmul out is capped at 512 elements.
  ACT exp [128,1024] f32(PSUM)->fp8: ~1.38us (0.9-1.0 ns/elem + ~290ns
  fixed + ~180ns PSUM read penalty). DVE copy [*,1024]: ~1.3us.

  Scores contract over D=64 only, so Q is stored ZERO-PADDED to 128
  partitions (per-head slot: valid 64 rows aligned with that head's rows
  in the packed K tile, zeros elsewhere) — the padded bf16 matmul runs at
  full rate, halving scores PE time with zero numerics change.

Dtypes: x/weights bf16, scores psum f32, exp out fp8e4m3, V fp8 (values
~e^|s| small), attn bf16, output f32. Offline numerics: rel err ~1.3e-2
(gate 2e-2).
"""
import os
import sys

sys.path.insert(0, "/opt/trn_rl_repo")

import numpy as np
import ml_dtypes

import concourse.bass as bass
import concourse.mybir as mybir
import concourse.tile as tile
from concourse import bacc
from concourse.bass_utils import run_bass_kernel_spmd

B, S, E, H, D = 4, 2048, 1024, 16, 64
HPC = 8            # heads per core
NPAIR = 4
NCORES = 8
P = 128
NST = S // P       # 16 s-tiles of 128
NSH = 2            # s-halves of 1024 per head
NSKP = NST // 2    # skt pairs
f32 = mybir.dt.float32
bf16 = mybir.dt.bfloat16
fp8 = mybir.dt.float8e4
AF = mybir.ActivationFunctionType
DR = mybir.MatmulPerfMode.DoubleRow
SCALE = 1.0 / 8.0  # 1/sqrt(D)

_BUILD_CACHE = {}
LAST_RESULTS = None

# timing-bisect flags (NOT correct output):
#  KERNEL_NOACT=1  — drop exp activations (PV reads a constant tile)
#  KERNEL_PEONLY=1 — NOACT + drop DVE/Pool/output work: bare matmul stream
NOACT = os.environ.get("KERNEL_NOACT", "0") == "1"
PEONLY = os.environ.get("KERNEL_PEONLY", "0") == "1"
if PEONLY:
    NOACT = True

# HW-measured PE costs (ns) for the filler pacing heuristic
A_COST, B_COST, D_COST = 940, 660, 235
CHUNK_BUDGET = 1320    # ACT per-skp time (2x1.38us) minus pinned PE work
BUDGET_CAP = 1700


def build_nc(repeat=1, bodies=1):
    PW = P  # fp8 DoubleRow ldweights wants full 128-column weight planes
    nc = bacc.Bacc("TRN2", target_bir_lowering=False, debug=False,
                   num_devices=NCORES)

    xT = nc.dram_tensor("xT", [E, S], bf16, kind="ExternalInput").ap()
    w_qk = nc.dram_tensor("w_qk", [E, HPC * P], bf16, kind="ExternalInput").ap()
    b_qk = nc.dram_tensor("b_qk", [HPC * P, 1], f32, kind="ExternalInput").ap()
    w_v = nc.dram_tensor("w_v", [E, HPC * D], bf16, kind="ExternalInput").ap()
    b_v = nc.dram_tensor("b_v", [P, HPC * D], f32, kind="ExternalInput").ap()
    w_out = nc.dram_tensor("w_out", [HPC * D, E], bf16, kind="ExternalInput").ap()
    outT = nc.dram_tensor("outT", [NPAIR, E, S], bf16, kind="ExternalOutput").ap()
    # DRAM bounce scratch for the denominator reshape [1,1024]<->[128,8]
    scrA = nc.dram_tensor("scrA", [16, 1024], f32, kind="Internal").ap()
    scrB = nc.dram_tensor("scrB", [16, 1024], f32, kind="Internal").ap()

    xT_r = xT.rearrange("(ko p) s -> p ko s", p=P)          # [128, 8, S]
    wqk_r = w_qk.rearrange("(ko p) f -> p ko f", p=P)       # [128, 8, 1024]
    wv_r = w_v.rearrange("(ko p) f -> p ko f", p=P)         # [128, 8, 512]
    bqk_r = b_qk.rearrange("(m p) one -> p (m one)", p=P)   # [128, 8]
    bv_r = b_v.rearrange("p (h d) -> p h d", d=D)           # [128, 8, 64]
    wo_r = w_out.rearrange("(j p) f -> p j f", p=P)         # [128, 4, 1024]
    outT_r = outT.rearrange("u (m p) s -> p u m s", p=P)    # [128, 4, 8, S]
    scrA_r = scrA.rearrange("f (p e) -> p f e", p=P)        # [128, 16, 8]
    scrB_r = scrB.rearrange("f (p e) -> p f e", p=P)        # [128, 16, 8]

    with tile.TileContext(nc) as tc:
        def body():
            from contextlib import ExitStack
            with ExitStack() as outer:
                persist = outer.enter_context(tc.tile_pool(name="persist", bufs=1))
                xsb = persist.tile([P, 8, S], bf16)
                wqk_sb = persist.tile([P, 8, HPC * P], bf16)
                wv_sb = persist.tile([P, 8, HPC * D], bf16)
                wo_sb = persist.tile([P, NPAIR, E], bf16)
                bqk_sb = persist.tile([P, 8], f32)
                bv_sb = persist.tile([P, HPC, D], f32)
                # Q per-head zero-padded to 128 partitions: slot h holds head
                # h's q on the 64 partitions matching its rows in the packed
                # K tile (even h -> 0:64, odd h -> 64:128), zeros elsewhere,
                # so the contract-128 scores matmul streams at full rate.
                qPAD = persist.tile([P, HPC, S], bf16)
                kT2 = persist.tile([P, NPAIR, S], bf16)
                # V with ones column (softmax denominators), zero-padded to
                # 128-wide planes for the dual-fp8 DoubleRow ldweights check.
                # dims [part, skp, head, skt-parity, 128], planes adjacent
                v_sb = persist.tile([P, NSKP, HPC, 2, PW], fp8)

                attnT_pool = outer.enter_context(
                    tc.tile_pool(name="attnT", bufs=NPAIR))
                pT_pool = outer.enter_context(tc.tile_pool(name="pT", bufs=8))
                rec_pool = outer.enter_context(tc.tile_pool(name="rec", bufs=2))
                r_pool = outer.enter_context(tc.tile_pool(name="r128", bufs=2))
                bc_pool = outer.enter_context(tc.tile_pool(name="bc", bufs=2))
                osb_pool = outer.enter_context(tc.tile_pool(name="osb", bufs=6))
                psS = outer.enter_context(
                    tc.tile_pool(name="psS", bufs=2, space="PSUM"))
                psA = outer.enter_context(
                    tc.tile_pool(name="psA", bufs=1, space="PSUM"))
                psX = outer.enter_context(
                    tc.tile_pool(name="psX", bufs=2, space="PSUM"))

                # ---- input DMAs: biases + x chunk 0 + w_qk first (QK proj
                # of pair 0 gates everything); k-halves split for earlier
                # start
                nc.sync.dma_start(bqk_sb[:], bqk_r)
                nc.sync.dma_start(xsb[:, 0:4, 0:512], xT_r[:, 0:4, 0:512])
                nc.sync.dma_start(wqk_sb[:, 0:4, :], wqk_r[:, 0:4, :])
                nc.sync.dma_start(xsb[:, 4:8, 0:512], xT_r[:, 4:8, 0:512])
                nc.sync.dma_start(wqk_sb[:, 4:8, :], wqk_r[:, 4:8, :])
                nc.sync.dma_start(xsb[:, :, 512:1024], xT_r[:, :, 512:1024])
                # second DMA queue (scalar engine, idle in prologue) for the
                # non-critical loads
                nc.scalar.dma_start(wv_sb[:], wv_r)
                nc.scalar.dma_start(bv_sb[:], bv_r)
                for q in range(2, 4):
                    sq = slice(q * 512, (q + 1) * 512)
                    nc.scalar.dma_start(xsb[:, :, sq], xT_r[:, :, sq])
                nc.scalar.dma_start(wo_sb[:], wo_r)
                nc.gpsimd.memset(v_sb[:, :, :, :, D:D + 1], 1.0)
                nc.gpsimd.memset(v_sb[:, :, :, :, D + 1:PW], 0.0)
                # zero the complementary halves of the padded-Q slots (never
                # written again; the zeros select one head in the packed K)
                for h in range(HPC):
                    if h % 2 == 0:
                        nc.gpsimd.memset(qPAD[64:128, h, :], 0.0)
                    else:
                        nc.gpsimd.memset(qPAD[0:64, h, :], 0.0)
                noact_pt = None
                if NOACT:
                    noact_pt = persist.tile([P, 2, 1024], fp8)
                    nc.gpsimd.memset(noact_pt[:], 0.25)
                # preload the Exp activation table off the critical path
                warm = rec_pool.tile([1, 1024], f32)
                nc.vector.memset(warm[0:1, 0:2], 0.0)
                nc.scalar.activation(warm[0:1, 0:2], warm[0:1, 0:2], AF.Exp)
                # warm the PE p-state during the DMA wait: ~3us of dummy
                # matmuls on the first-arrived tile (outputs never read)
                wps = psX.tile([8, 8], f32, name="pwarm", tag="psx")
                for _ in range(12):
                    nc.tensor.matmul(wps[:], lhsT=bqk_sb[:, 0:8],
                                     rhs=bqk_sb[:, 0:8], start=True, stop=True)

                # ---- B quantum: V projection for one (s-tile, head pair)
                def b_quant(st, pr):
                    ps = psX.tile([P, 2 * D], f32, name="psb", tag="psx")
                    for k in range(8):
                        nc.tensor.matmul(
                            ps[:],
                            lhsT=xsb[:, k, st * P:(st + 1) * P],
                            rhs=wv_sb[:, k, pr * 2 * D:(pr + 1) * 2 * D],
                            start=(k == 0), stop=(k == 7))
                    if not PEONLY:
                        nc.vector.tensor_add(
                            v_sb[:, st // 2, 2 * pr:2 * pr + 2, st % 2, 0:D],
                            ps.rearrange("p (h d) -> p h d", d=D),
                            bv_sb[:, 2 * pr:2 * pr + 2, :])

                # ---- A quanta: QK projection for one (m-tile, q-chunk)
                open_psa = {}

                def a_quant(m, q, half):
                    sq = slice(q * 512, (q + 1) * 512)
                    if half == 0:
                        ps = psX.tile([P, 512], f32, name="psa", tag="psx")
                        open_psa[(m, q)] = ps
                    else:
                        ps = open_psa.pop((m, q))
                    for k in range(4 * half, 4 * half + 4):
                        nc.tensor.matmul(
                            ps[:], lhsT=wqk_sb[:, k, m * P:(m + 1) * P],
                            rhs=xsb[:, k, sq],
                            start=(k == 0), stop=(k == 7))
                    if half == 1 and not PEONLY:
                        if m % 2 == 0:
                            # q of pair u=m//2: two half-partition writes into
                            # the per-head padded slots
                            u = m // 2
                            nc.vector.tensor_scalar_add(
                                qPAD[0:64, 2 * u, sq], ps[0:64, :],
                                bqk_sb[0:64, m:m + 1])
                            nc.vector.tensor_scalar_add(
                                qPAD[64:128, 2 * u + 1, sq], ps[64:128, :],
                                bqk_sb[64:128, m:m + 1])
                        else:
                            nc.vector.tensor_scalar_add(
                                kT2[:, m // 2, sq], ps[:], bqk_sb[:, m:m + 1])

                # Minimal inline prologue: first scores chunk needs K s-tiles
                # 0-3 (K q-chunk 0) and Q cols 0:1024 (Q q-chunks 0,1)
                inline_tags = set()
                for m, q in ((1, 0), (0, 0), (0, 1)):
                    a_quant(m, q, 0)
                    a_quant(m, q, 1)
                    inline_tags |= {("A", 0, m, q, 0), ("A", 0, m, q, 1)}

                # ---- filler queue: (tag, PE-cost-ns, closure)
                # tags: ("A", pair, m, q, half) / ("B", pair, st) / ("D",...)
                fillers = []
                emitted = set(inline_tags)

                def _run(f):
                    tag, _, fn = f
                    fn()
                    emitted.add(tag)

                pool_ns = [0.0]

                def drain_budget(ns):
                    pool_ns[0] = min(pool_ns[0] + ns, float(BUDGET_CAP))
                    while fillers and fillers[0][1] <= pool_ns[0]:
                        f = fillers.pop(0)
                        pool_ns[0] -= f[1]
                        _run(f)

                def need(tag):
                    while fillers and tag not in emitted:
                        _run(fillers.pop(0))

                def af(m, q, h):
                    return (("A", m // 2, m, q, h), A_COST,
                            lambda: a_quant(m, q, h))

                def bf(st, pr):
                    return (("B", pr, st), B_COST, lambda: b_quant(st, pr))

                # deferred prologue, ordered so head 0's consumption paces:
                # V(st) needed by PV(skp=st//2), K q-chunk c by scores skt>=4c,
                # Q q-chunks 2,3 only by s-half 1
                front = []
                front += [bf(st, 0) for st in (0, 1)]
                front += [af(1, 1, h) for h in (0, 1)]
                front += [bf(st, 0) for st in (2, 3)]
                front += [af(1, 2, h) for h in (0, 1)]
                front += [bf(st, 0) for st in (4, 5)]
                front += [af(1, 3, h) for h in (0, 1)]
                front += [bf(st, 0) for st in range(6, NST)]
                for q in (2, 3):
                    front += [af(0, q, h) for h in (0, 1)]
                fillers.extend(front)

                def d_quant(u, attnT_u, m, c, evac="dve", pool=None):
                    sc = slice(c * 512, (c + 1) * 512)
                    if pool is None:
                        ps = psX.tile([P, 512], f32, name="psd", tag="psx")
                    else:
                        ps = pool.tile([P, 1024], f32, name="ps_s",
                                       tag="ps_s")[:, 0:512]
                    nc.tensor.matmul(
                        ps[:], lhsT=wo_sb[:, u, m * P:(m + 1) * P],
                        rhs=attnT_u[:, sc], start=True, stop=True)
                    if PEONLY:
                        return
                    o = osb_pool.tile([P, 512], bf16)
                    if evac == "act":
                        nc.scalar.copy(o[:], ps[:])
                    else:
                        nc.vector.tensor_copy(o[:], ps[:])
                    nc.sync.dma_start(outT_r[:, u, m, sc], o[:])

                # ---- main loop over head pairs
                pending = [None]

                def flush_pending():
                    if pending[0] is not None:
                        pvfn, fin = pending[0]
                        pending[0] = None
                        pvfn()
                        if fin is not None:
                            fin()

                attnT_tiles = {}
                for u in range(NPAIR):
                    attnT_u = attnT_pool.tile([P, S], bf16)
                    attnT_tiles[u] = attnT_u
                    if PEONLY:
                        nc.gpsimd.memset(attnT_u[:], 0.25)

                    # prefetch fillers for pair u+1, ordered like the pair-0
                    # front so need() rarely has to force anything. D quanta
                    # for pair u-1 go behind (no deadline until the tail).
                    if u + 1 < NPAIR:
                        un = u + 1
                        mq, mk = 2 * un, 2 * un + 1
                        pf = []
                        pf += [af(mk, 0, h) for h in (0, 1)]
                        pf += [af(mq, 0, h) for h in (0, 1)]
                        pf += [af(mq, 1, h) for h in (0, 1)]
                        pf += [bf(st, un) for st in (0, 1)]
                        pf += [af(mk, 1, h) for h in (0, 1)]
                        pf += [bf(st, un) for st in (2, 3)]
                        pf += [af(mk, 2, h) for h in (0, 1)]
                        pf += [bf(st, un) for st in (4, 5)]
                        pf += [af(mk, 3, h) for h in (0, 1)]
                        pf += [bf(st, un) for st in range(6, NST)]
                        pf += [af(mq, 2, h) for h in (0, 1)]
                        pf += [af(mq, 3, h) for h in (0, 1)]
                        fillers.extend(pf)
                    if u > 0:
                        up, at_p = u - 1, attnT_tiles[u - 1]
                        for m in range(8):
                            for c in range(4):
                                fillers.append(
                                    (("D", up, m, c), D_COST,
                                     (lambda m=m, c=c, up=up, at_p=at_p:
                                      d_quant(up, at_p, m, c))))

                    for par in range(2):
                        i = 2 * u + par
                        poff = 64 * par
                        for sh in range(NSH):
                            s0 = sh * 1024
                            need(("A", u, 2 * u, 2 * sh, 1))
                            need(("A", u, 2 * u, 2 * sh + 1, 1))
                            at = psA.tile([P, 1024], f32)

                            def mk_pv(skp, pt, at=at, i=i, u=u):
                                def pv():
                                    need(("B", u, 2 * skp + 1))
                                    for c in range(2):
                                        sc = slice(c * 512, (c + 1) * 512)
                                        nc.tensor.matmul(
                                            at[:, sc],
                                            lhsT=v_sb[:, skp, i, :, :],
                                            rhs=pt[:, :, sc],
                                            start=(skp == 0),
                                            stop=(skp == NSKP - 1),
                                            perf_mode=DR)
                                return pv

                            def mk_fin(at=at, i=i, u=u, par=par, sh=sh,
                                       poff=poff, s0=s0, attnT_u=attnT_u):
                                def fin(fin_idx=4 * u + 2 * par + sh):
                                    if PEONLY:
                                        if u == NPAIR - 1 and par == 1:
                                            tailq = []
                                            for c in (2 * sh, 2 * sh + 1):
                                                for m in range(8):
                                                    tailq.append(
                                                        (("D", u, m, c),
                                                         D_COST,
                                                         (lambda m=m, c=c,
                                                          at_u=attnT_u:
                                                          d_quant(NPAIR - 1,
                                                                  at_u, m,
                                                                  c))))
                                            fillers[:0] = tailq
                                        return
                                    # denominators -> reciprocal; evacuate
                                    # values; normalize via Pool broadcast.
                                    # A [1,1024] DVE reciprocal runs on ONE
                                    # lane (6.35us measured); bounce the 4KB
                                    # row through DRAM to [128,8] so all
                                    # lanes work (~0.1us), then bounce back.
                                    # All 4 hops share the scalar DMA queue
                                    # whose FIFO orders the DRAM RAW hazards.
                                    rec = rec_pool.tile([1, 1024], f32)
                                    den_sb = rec_pool.tile([1, 1024], f32)
                                    r128 = r_pool.tile([P, 8], f32)
                                    r128b = r_pool.tile([P, 8], f32)
                                    # PSUM-source DMA is unsupported: hop to
                                    # SBUF on the (slack) ACT engine first
                                    nc.scalar.copy(den_sb[:], at[D:D + 1, :])
                                    nc.scalar.dma_start(
                                        scrA[fin_idx:fin_idx + 1, :],
                                        den_sb[:])
                                    nc.scalar.dma_start(
                                        r128[:], scrA_r[:, fin_idx, :])
                                    nc.vector.reciprocal(r128b[:], r128[:])
                                    nc.scalar.dma_start(
                                        scrB_r[:, fin_idx, :], r128b[:])
                                    nc.scalar.dma_start(
                                        rec[:], scrB[fin_idx:fin_idx + 1, :])
                                    nc.vector.tensor_copy(
                                        attnT_u[poff:poff + 64,
                                                s0:s0 + 1024],
                                        at[0:D, :])
                                    bc = bc_pool.tile([P, 1024], f32)
                                    nc.gpsimd.partition_broadcast(
                                        bc[:], rec[:], P)
                                    nc.gpsimd.tensor_mul(
                                        attnT_u[poff:poff + 64,
                                                s0:s0 + 1024],
                                        attnT_u[poff:poff + 64,
                                                s0:s0 + 1024],
                                        bc[poff:poff + 64, :])
                                    if u == NPAIR - 1 and par == 1:
                                        # last pair: tail D quanta, DVE/ACT
                                        # split, borrowing free score banks
                                        tailq = []
                                        for c in (2 * sh, 2 * sh + 1):
                                            for m in range(8):
                                                ev = "act" if (sh == 1 and
                                                               m % 2) \
                                                    else "dve"
                                                pl = psS if (
                                                    sh == 1 and
                                                    (m // 2) % 2) else None
                                                tailq.append(
                                                    (("D", u, m, c), D_COST,
                                                     (lambda m=m, c=c,
                                                      at_u=attnT_u, ev=ev,
                                                      pl=pl:
                                                      d_quant(NPAIR - 1,
                                                              at_u, m, c,
                                                              ev, pl))))
                                        fillers[:0] = tailq
                                return fin

                            for skp in range(NSKP):
                                pt = noact_pt if NOACT else \
                                    pT_pool.tile([P, 2, 1024], fp8)
                                for sp in range(2):
                                    skt = 2 * skp + sp
                                    need(("A", u, 2 * u + 1, skt // 4, 1))
                                    ps_s = psS.tile([P, 1024], f32,
                                                    name="ps_s", tag="ps_s")
                                    for c in range(2):
                                        # contract-128: packed K (both heads)
                                        # x zero-padded per-head Q slot
                                        nc.tensor.matmul(
                                            ps_s[:, c * 512:(c + 1) * 512],
                                            lhsT=kT2[:, u, skt * P:
                                                     (skt + 1) * P],
                                            rhs=qPAD[:, i,
                                                     s0 + c * 512:
                                                     s0 + (c + 1) * 512],
                                            start=True, stop=True)
                                    if not NOACT:
                                        nc.scalar.activation(
                                            pt[:, sp, :], ps_s[:], AF.Exp,
                                            scale=SCALE)
                                # the previous chunk's PV (possibly from the
                                # previous s-half/head) flushes only after
                                # this chunk's scores+exp feed the ACT engine
                                flush_pending()
                                pending[0] = (
                                    mk_pv(skp, pt),
                                    mk_fin() if skp == NSKP - 1 else None)
                                drain_budget(CHUNK_BUDGET)
                flush_pending()
                while fillers:
                    _run(fillers.pop(0))

        if repeat > 1:
            with tc.For_i(0, repeat, 1):
                for _ in range(bodies):
                    body()
        else:
            body()

    nc.compile()
    return nc


def _get_nc(repeat=1, bodies=1):
    key = (repeat, bodies)
    if key not in _BUILD_CACHE:
        _BUILD_CACHE[key] = build_nc(repeat=repeat, bodies=bodies)
    return _BUILD_CACHE[key]


def shard_inputs(x, w_qkv, b_qkv, w_out, b_out):
    """Host-side sharding: per-core input maps."""
    bf = ml_dtypes.bfloat16
    in_maps = []
    for c in range(NCORES):
        b, g = c // 2, c % 2
        heads = [g * HPC + i for i in range(HPC)]
        # qk columns, pair-interleaved: m-tile 2u = q cols of pair u,
        # m-tile 2u+1 = k cols of pair u; within a tile [headA 64 | headB 64]
        qk_cols, qk_bias = [], []
        for u in range(HPC // 2):
            hA, hB = heads[2 * u], heads[2 * u + 1]
            for off in (0, 64):  # 0: q, 64: k
                for h in (hA, hB):
                    qk_cols.append(w_qkv[:, h * 192 + off:h * 192 + off + 64])
                    qk_bias.append(b_qkv[h * 192 + off:h * 192 + off + 64])
        w_qk_c = np.ascontiguousarray(
            np.concatenate(qk_cols, axis=1)).astype(bf)
        b_qk_c = np.ascontiguousarray(
            np.concatenate(qk_bias)[:, None].astype(np.float32))
        w_v_c = np.ascontiguousarray(np.concatenate(
            [w_qkv[:, h * 192 + 128:h * 192 + 192] for h in heads],
            axis=1)).astype(bf)
        b_v_c = np.ascontiguousarray(np.broadcast_to(np.concatenate(
            [b_qkv[h * 192 + 128:h * 192 + 192] for h in heads])[None, :],
            (P, HPC * D)).astype(np.float32))
        w_out_c = np.ascontiguousarray(np.concatenate(
            [w_out[h * D:(h + 1) * D, :] for h in heads], axis=0)).astype(bf)
        xT_c = np.ascontiguousarray(x[b].T).astype(bf)
        in_maps.append({
            "xT": xT_c, "w_qk": w_qk_c, "b_qk": b_qk_c,
            "w_v": w_v_c, "b_v": b_v_c, "w_out": w_out_c,
        })
    return in_maps


def unshard_output(results, b_out):
    out = np.empty((B, S, E), dtype=np.float32)
    for b in range(B):
        acc = results[2 * b]["outT"].astype(np.float32).sum(axis=0)
        acc += results[2 * b + 1]["outT"].astype(np.float32).sum(axis=0)
        out[b] = acc.T + b_out
    return out


def kernel(x, w_qkv, b_qkv, w_out, b_out):
    global LAST_RESULTS
    x = np.asarray(x, dtype=np.float32)
    w_qkv = np.asarray(w_qkv, dtype=np.float32)
    b_qkv = np.asarray(b_qkv, dtype=np.float32)
    w_out = np.asarray(w_out, dtype=np.float32)
    b_out = np.asarray(b_out, dtype=np.float32)

    nc = _get_nc()
    in_maps = shard_inputs(x, w_qkv, b_qkv, w_out, b_out)
    try:
        res = run_bass_kernel_spmd(nc, in_maps, list(range(NCORES)))
    except ModuleNotFoundError:
        # BASS_TRACE requested but this axon client has no NTFF hook module
        os.environ["BASS_NEVER_TRACE"] = "1"
        res = run_bass_kernel_spmd(nc, in_maps, list(range(NCORES)))
    LAST_RESULTS = res
    return unshard_output(res.results, b_out)


# revision 19
# speedup vs baseline: 1.2590x; 1.0397x over previous
"""Multi-head attention (B=4, S=2048, E=1024, H=16, D=64) on 8 Trainium2 cores.

Sharding: core c handles batch b=c//2 and head-group g=c%2 (8 heads, 4 pairs).

Per-core schedule (single fused stream; ACT(exp)-bound on HW):
  prologue: V = x @ w_v (+bias) for all heads; QK proj for pair 0.
  for pair u: for head, for s-half: pipelined scores (bf16 matmul) ->
    exp (scalar engine, fp8 out) -> P@V (fp8 DoubleRow matmul, ones column
    gives softmax denominators). QK proj of pair u+1 and output proj of
    pair u-1 are drained into the PE stream as filler quanta so the tensor
    engine never idles while the scalar engine works through the exps.
  Normalization per (head, s-half): reciprocal of denominators (DVE),
    partition-broadcast (gpsimd), multiply on gpsimd.
  Output proj per pair -> PSUM -> SBUF -> DRAM partial; host sums the
  4 pair-partials x 2 head-group cores per batch and adds b_out.

HW calibration (measured via engine-pure probes, wall-clock-delta):
  PE matmul streams ~1 col/cycle @2.4GHz ONLY when lhsT occupies 128
  physical partitions; 64-partition weights run at HALF rate (and fp8
  DoubleRow with 64-partition 2-plane weights is half rate too — the rate
  follows physical partitions). Matmul out is capped at 512 elements.
  ACT exp [128,1024] f32(PSUM)->fp8: ~1.38us (0.9-1.0 ns/elem + ~290ns
  fixed + ~180ns PSUM read penalty). DVE copy [*,1024]: ~1.3us.

  Scores contract over D=64 only, so Q is stored ZERO-PADDED to 128
  partitions (per-head slot: valid 64 rows aligned with that head's rows
  in the packed K tile, zeros elsewhere) — the padded bf16 matmul runs at
  full rate, halving scores PE time with zero numerics change.

Dtypes: x/weights bf16, scores psum f32, exp out fp8e4m3, V fp8 (values
~e^|s| small), attn bf16, output f32. Offline numerics: rel err ~1.3e-2
(gate 2e-2).
"""
import os
import sys

sys.path.insert(0, "/opt/trn_rl_repo")

import numpy as np
import ml_dtypes

import concourse.bass as bass
import concourse.mybir as mybir
import concourse.tile as tile
from concourse import bacc
from concourse.bass_utils import run_bass_kernel_spmd

B, S, E, H, D = 4, 2048, 1024, 16, 64
HPC = 8            # heads per core
NPAIR = 4
NCORES = 8
P = 128
NST = S // P       # 16 s-tiles of 128
NSH = 2            # s-halves of 1024 per head
NSKP = NST // 2    # skt pairs
f32 = mybir.dt.float32
bf16 = mybir.dt.bfloat16
fp8 = mybir.dt.float8e4
AF = mybir.ActivationFunctionType
DR = mybir.MatmulPerfMode.DoubleRow
SCALE = 1.0 / 8.0  # 1/sqrt(D)

_BUILD_CACHE = {}
LAST_RESULTS = None

# timing-bisect flags (NOT correct output):
#  KERNEL_NOACT=1  — drop exp activations (PV reads a constant tile)
#  KERNEL_PEONLY=1 — NOACT + drop DVE/Pool/output work: bare matmul stream
NOACT = os.environ.get("KERNEL_NOACT", "0") == "1"
PEONLY = os.environ.get("KERNEL_PEONLY", "0") == "1"
if PEONLY:
    NOACT = True

# HW-measured PE costs (ns) for the filler pacing heuristic
A_COST, B_COST, D_COST = 940, 660, 235
CHUNK_BUDGET = 1320    # ACT per-skp time (2x1.38us) minus pinned PE work
BUDGET_CAP = 1700


def build_nc(repeat=1, bodies=1):
    PW = P  # fp8 DoubleRow ldweights wants full 128-column weight planes
    nc = bacc.Bacc("TRN2", target_bir_lowering=False, debug=False,
                   num_devices=NCORES)

    xT = nc.dram_tensor("xT", [E, S], bf16, kind="ExternalInput").ap()
    w_qk = nc.dram_tensor("w_qk", [E, HPC * P], bf16, kind="ExternalInput").ap()
    b_qk = nc.dram_tensor("b_qk", [HPC * P, 1], f32, kind="ExternalInput").ap()
    w_v = nc.dram_tensor("w_v", [E, HPC * D], bf16, kind="ExternalInput").ap()
    b_v = nc.dram_tensor("b_v", [P, HPC * D], f32, kind="ExternalInput").ap()
    w_out = nc.dram_tensor("w_out", [HPC * D, E], bf16, kind="ExternalInput").ap()
    outT = nc.dram_tensor("outT", [NPAIR, E, S], bf16, kind="ExternalOutput").ap()
    # DRAM bounce scratch for the denominator reshape [1,1024]<->[128,8]
    scrA = nc.dram_tensor("scrA", [16, 1024], f32, kind="Internal").ap()
    scrB = nc.dram_tensor("scrB", [16, 1024], f32, kind="Internal").ap()

    xT_r = xT.rearrange("(ko p) s -> p ko s", p=P)          # [128, 8, S]
    wqk_r = w_qk.rearrange("(ko p) f -> p ko f", p=P)       # [128, 8, 1024]
    wv_r = w_v.rearrange("(ko p) f -> p ko f", p=P)         # [128, 8, 512]
    bqk_r = b_qk.rearrange("(m p) one -> p (m one)", p=P)   # [128, 8]
    bv_r = b_v.rearrange("p (h d) -> p h d", d=D)           # [128, 8, 64]
    wo_r = w_out.rearrange("(j p) f -> p j f", p=P)         # [128, 4, 1024]
    outT_r = outT.rearrange("u (m p) s -> p u m s", p=P)    # [128, 4, 8, S]
    scrA_r = scrA.rearrange("f (p e) -> p f e", p=P)        # [128, 16, 8]
    scrB_r = scrB.rearrange("f (p e) -> p f e", p=P)        # [128, 16, 8]

    with tile.TileContext(nc) as tc:
        def body():
            from contextlib import ExitStack
            with ExitStack() as outer:
                persist = outer.enter_context(tc.tile_pool(name="persist", bufs=1))
                xsb = persist.tile([P, 8, S], bf16)
                wqk_sb = persist.tile([P, 8, HPC * P], bf16)
                wv_sb = persist.tile([P, 8, HPC * D], bf16)
                wo_sb = persist.tile([P, NPAIR, E], bf16)
                bqk_sb = persist.tile([P, 8], f32)
                bv_sb = persist.tile([P, HPC, D], f32)
                # Q per-head zero-padded to 128 partitions: slot h holds head
                # h's q on the 64 partitions matching its rows in the packed
                # K tile (even h -> 0:64, odd h -> 64:128), zeros elsewhere,
                # so the contract-128 scores matmul streams at full rate.
                qPAD = persist.tile([P, HPC, S], bf16)
                kT2 = persist.tile([P, NPAIR, S], bf16)
                # V with ones column (softmax denominators), zero-padded to
                # 128-wide planes for the dual-fp8 DoubleRow ldweights check.
                # dims [part, skp, head, skt-parity, 128], planes adjacent
                v_sb = persist.tile([P, NSKP, HPC, 2, PW], fp8)

                attnT_pool = outer.enter_context(
                    tc.tile_pool(name="attnT", bufs=NPAIR))
                pT_pool = outer.enter_context(tc.tile_pool(name="pT", bufs=8))
                rec_pool = outer.enter_context(tc.tile_pool(name="rec", bufs=2))
                r_pool = outer.enter_context(tc.tile_pool(name="r128", bufs=2))
                bc_pool = outer.enter_context(tc.tile_pool(name="bc", bufs=2))
                osb_pool = outer.enter_context(tc.tile_pool(name="osb", bufs=6))
                psS = outer.enter_context(
                    tc.tile_pool(name="psS", bufs=2, space="PSUM"))
                psA = outer.enter_context(
                    tc.tile_pool(name="psA", bufs=1, space="PSUM"))
                psX = outer.enter_context(
                    tc.tile_pool(name="psX", bufs=2, space="PSUM"))

                # ---- input DMAs: biases + x chunk 0 + w_qk first (QK proj
                # of pair 0 gates everything); k-halves split for earlier
                # start
                nc.sync.dma_start(bqk_sb[:], bqk_r)
                nc.sync.dma_start(xsb[:, 0:4, 0:512], xT_r[:, 0:4, 0:512])
                nc.sync.dma_start(wqk_sb[:, 0:4, :], wqk_r[:, 0:4, :])
                nc.sync.dma_start(xsb[:, 4:8, 0:512], xT_r[:, 4:8, 0:512])
                nc.sync.dma_start(wqk_sb[:, 4:8, :], wqk_r[:, 4:8, :])
                nc.sync.dma_start(xsb[:, :, 512:1024], xT_r[:, :, 512:1024])
                # second DMA queue (scalar engine, idle in prologue) for the
                # non-critical loads
                nc.scalar.dma_start(wv_sb[:], wv_r)
                nc.scalar.dma_start(bv_sb[:], bv_r)
                for q in range(2, 4):
                    sq = slice(q * 512, (q + 1) * 512)
                    nc.scalar.dma_start(xsb[:, :, sq], xT_r[:, :, sq])
                nc.scalar.dma_start(wo_sb[:], wo_r)
                nc.gpsimd.memset(v_sb[:, :, :, :, D:D + 1], 1.0)
                nc.gpsimd.memset(v_sb[:, :, :, :, D + 1:PW], 0.0)
                # zero the complementary halves of the padded-Q slots (never
                # written again; the zeros select one head in the packed K)
                for h in range(HPC):
                    if h % 2 == 0:
                        nc.gpsimd.memset(qPAD[64:128, h, :], 0.0)
                    else:
                        nc.gpsimd.memset(qPAD[0:64, h, :], 0.0)
                noact_pt = None
                if NOACT:
                    noact_pt = persist.tile([P, 2, 1024], fp8)
                    nc.gpsimd.memset(noact_pt[:], 0.25)
                # preload the Exp activation table off the critical path
                warm = rec_pool.tile([1, 1024], f32)
                nc.vector.memset(warm[0:1, 0:2], 0.0)
                nc.scalar.activation(warm[0:1, 0:2], warm[0:1, 0:2], AF.Exp)
                # warm the PE p-state during the DMA wait: ~3us of dummy
                # matmuls on the first-arrived tile (outputs never read)
                wps = psX.tile([8, 8], f32, name="pwarm", tag="psx")
                for _ in range(12):
                    nc.tensor.matmul(wps[:], lhsT=bqk_sb[:, 0:8],
                                     rhs=bqk_sb[:, 0:8], start=True, stop=True)

                # ---- B quantum: V projection for one (s-tile, head pair)
                def b_quant(st, pr):
                    ps = psX.tile([P, 2 * D], f32, name="psb", tag="psx")
                    for k in range(8):
                        nc.tensor.matmul(
                            ps[:],
                            lhsT=xsb[:, k, st * P:(st + 1) * P],
                            rhs=wv_sb[:, k, pr * 2 * D:(pr + 1) * 2 * D],
                            start=(k == 0), stop=(k == 7))
                    if not PEONLY:
                        nc.vector.tensor_add(
                            v_sb[:, st // 2, 2 * pr:2 * pr + 2, st % 2, 0:D],
                            ps.rearrange("p (h d) -> p h d", d=D),
                            bv_sb[:, 2 * pr:2 * pr + 2, :])

                # ---- A quanta: QK projection for one (m-tile, q-chunk)
                open_psa = {}

                def a_quant(m, q, half):
                    sq = slice(q * 512, (q + 1) * 512)
                    if half == 0:
                        ps = psX.tile([P, 512], f32, name="psa", tag="psx")
                        open_psa[(m, q)] = ps
                    else:
                        ps = open_psa.pop((m, q))
                    for k in range(4 * half, 4 * half + 4):
                        nc.tensor.matmul(
                            ps[:], lhsT=wqk_sb[:, k, m * P:(m + 1) * P],
                            rhs=xsb[:, k, sq],
                            start=(k == 0), stop=(k == 7))
                    if half == 1 and not PEONLY:
                        if m % 2 == 0:
                            # q of pair u=m//2: two half-partition writes into
                            # the per-head padded slots
                            u = m // 2
                            nc.vector.tensor_scalar_add(
                                qPAD[0:64, 2 * u, sq], ps[0:64, :],
                                bqk_sb[0:64, m:m + 1])
                            nc.vector.tensor_scalar_add(
                                qPAD[64:128, 2 * u + 1, sq], ps[64:128, :],
                                bqk_sb[64:128, m:m + 1])
                        else:
                            nc.vector.tensor_scalar_add(
                                kT2[:, m // 2, sq], ps[:], bqk_sb[:, m:m + 1])

                # Minimal inline prologue: first scores chunk needs K s-tiles
                # 0-3 (K q-chunk 0) and Q cols 0:1024 (Q q-chunks 0,1)
                inline_tags = set()
                for m, q in ((1, 0), (0, 0), (0, 1)):
                    a_quant(m, q, 0)
                    a_quant(m, q, 1)
                    inline_tags |= {("A", 0, m, q, 0), ("A", 0, m, q, 1)}

                # ---- filler queue: (tag, PE-cost-ns, closure)
                # tags: ("A", pair, m, q, half) / ("B", pair, st) / ("D",...)
                fillers = []
                emitted = set(inline_tags)

                def _run(f):
                    tag, _, fn = f
                    fn()
                    emitted.add(tag)

                pool_ns = [0.0]

                def drain_budget(ns):
                    pool_ns[0] = min(pool_ns[0] + ns, float(BUDGET_CAP))
                    while fillers and fillers[0][1] <= pool_ns[0]:
                        f = fillers.pop(0)
                        pool_ns[0] -= f[1]
                        _run(f)

                def need(tag):
                    while fillers and tag not in emitted:
                        _run(fillers.pop(0))

                def af(m, q, h):
                    return (("A", m // 2, m, q, h), A_COST,
                            lambda: a_quant(m, q, h))

                def bf(st, pr):
                    return (("B", pr, st), B_COST, lambda: b_quant(st, pr))

                # deferred prologue, ordered so head 0's consumption paces:
                # V(st) needed by PV(skp=st//2), K q-chunk c by scores skt>=4c,
                # Q q-chunks 2,3 only by s-half 1
                front = []
                front += [bf(st, 0) for st in (0, 1)]
                front += [af(1, 1, h) for h in (0, 1)]
                front += [bf(st, 0) for st in (2, 3)]
                front += [af(1, 2, h) for h in (0, 1)]
                front += [bf(st, 0) for st in (4, 5)]
                front += [af(1, 3, h) for h in (0, 1)]
                front += [bf(st, 0) for st in range(6, NST)]
                for q in (2, 3):
                    front += [af(0, q, h) for h in (0, 1)]
                fillers.extend(front)

                def d_quant(u, attnT_u, m, c, evac="dve", pool=None):
                    sc = slice(c * 512, (c + 1) * 512)
                    if pool is None:
                        ps = psX.tile([P, 512], f32, name="psd", tag="psx")
                    else:
                        ps = pool.tile([P, 1024], f32, name="ps_s",
                                       tag="ps_s")[:, 0:512]
                    nc.tensor.matmul(
                        ps[:], lhsT=wo_sb[:, u, m * P:(m + 1) * P],
                        rhs=attnT_u[:, sc], start=True, stop=True)
                    if PEONLY:
                        return
                    o = osb_pool.tile([P, 512], bf16)
                    if evac == "act":
                        nc.scalar.copy(o[:], ps[:])
                    else:
                        nc.vector.tensor_copy(o[:], ps[:])
                    nc.sync.dma_start(outT_r[:, u, m, sc], o[:])

                # ---- main loop over head pairs
                pending = [None]

                def flush_pending():
                    if pending[0] is not None:
                        pvfn, fin = pending[0]
                        pending[0] = None
                        pvfn()
                        if fin is not None:
                            fin()

                attnT_tiles = {}
                for u in range(NPAIR):
                    attnT_u = attnT_pool.tile([P, S], bf16)
                    attnT_tiles[u] = attnT_u
                    if PEONLY:
                        nc.gpsimd.memset(attnT_u[:], 0.25)

                    # prefetch fillers for pair u+1, ordered like the pair-0
                    # front so need() rarely has to force anything. D quanta
                    # for pair u-1 go behind (no deadline until the tail).
                    if u + 1 < NPAIR:
                        un = u + 1
                        mq, mk = 2 * un, 2 * un + 1
                        pf = []
                        pf += [af(mk, 0, h) for h in (0, 1)]
                        pf += [af(mq, 0, h) for h in (0, 1)]
                        pf += [af(mq, 1, h) for h in (0, 1)]
                        pf += [bf(st, un) for st in (0, 1)]
                        pf += [af(mk, 1, h) for h in (0, 1)]
                        pf += [bf(st, un) for st in (2, 3)]
                        pf += [af(mk, 2, h) for h in (0, 1)]
                        pf += [bf(st, un) for st in (4, 5)]
                        pf += [af(mk, 3, h) for h in (0, 1)]
                        pf += [bf(st, un) for st in range(6, NST)]
                        pf += [af(mq, 2, h) for h in (0, 1)]
                        pf += [af(mq, 3, h) for h in (0, 1)]
                        fillers.extend(pf)
                    if u > 0:
                        up, at_p = u - 1, attnT_tiles[u - 1]
                        for m in range(8):
                            for c in range(4):
                                # odd m evacuate via ACT (slack there) to
                                # relieve the convoy-prone DVE queue
                                ev = "act" if m % 2 else "dve"
                                fillers.append(
                                    (("D", up, m, c), D_COST,
                                     (lambda m=m, c=c, up=up, at_p=at_p,
                                      ev=ev:
                                      d_quant(up, at_p, m, c, ev))))

                    for par in range(2):
                        i = 2 * u + par
                        poff = 64 * par
                        for sh in range(NSH):
                            s0 = sh * 1024
                            need(("A", u, 2 * u, 2 * sh, 1))
                            need(("A", u, 2 * u, 2 * sh + 1, 1))
                            at = psA.tile([P, 1024], f32)

                            def mk_pv(skp, pt, at=at, i=i, u=u):
                                def pv():
                                    need(("B", u, 2 * skp + 1))
                                    for c in range(2):
                                        sc = slice(c * 512, (c + 1) * 512)
                                        nc.tensor.matmul(
                                            at[:, sc],
                                            lhsT=v_sb[:, skp, i, :, :],
                                            rhs=pt[:, :, sc],
                                            start=(skp == 0),
                                            stop=(skp == NSKP - 1),
                                            perf_mode=DR)
                                return pv

                            def mk_fin(at=at, i=i, u=u, par=par, sh=sh,
                                       poff=poff, s0=s0, attnT_u=attnT_u):
                                def fin(fin_idx=4 * u + 2 * par + sh):
                                    if PEONLY:
                                        if u == NPAIR - 1 and par == 1:
                                            tailq = []
                                            for c in (2 * sh, 2 * sh + 1):
                                                for m in range(8):
                                                    tailq.append(
                                                        (("D", u, m, c),
                                                         D_COST,
                                                         (lambda m=m, c=c,
                                                          at_u=attnT_u:
                                                          d_quant(NPAIR - 1,
                                                                  at_u, m,
                                                                  c))))
                                            fillers[:0] = tailq
                                        return
                                    # denominators -> reciprocal; evacuate
                                    # values; normalize via Pool broadcast.
                                    # A [1,1024] DVE reciprocal runs on ONE
                                    # lane (6.35us measured); bounce the 4KB
                                    # row through DRAM to [128,8] so all
                                    # lanes work (~0.1us), then bounce back.
                                    # All 4 hops share the scalar DMA queue
                                    # whose FIFO orders the DRAM RAW hazards.
                                    rec = rec_pool.tile([1, 1024], f32)
                                    den_sb = rec_pool.tile([1, 1024], f32)
                                    r128 = r_pool.tile([P, 8], f32)
                                    r128b = r_pool.tile([P, 8], f32)
                                    # PSUM-source DMA is unsupported: hop to
                                    # SBUF on the (slack) ACT engine first
                                    nc.scalar.copy(den_sb[:], at[D:D + 1, :])
                                    nc.scalar.dma_start(
                                        scrA[fin_idx:fin_idx + 1, :],
                                        den_sb[:])
                                    nc.scalar.dma_start(
                                        r128[:], scrA_r[:, fin_idx, :])
                                    nc.vector.reciprocal(r128b[:], r128[:])
                                    nc.scalar.dma_start(
                                        scrB_r[:, fin_idx, :], r128b[:])
                                    nc.scalar.dma_start(
                                        rec[:], scrB[fin_idx:fin_idx + 1, :])
                                    nc.vector.tensor_copy(
                                        attnT_u[poff:poff + 64,
                                                s0:s0 + 1024],
                                        at[0:D, :])
                                    bc = bc_pool.tile([P, 1024], f32)
                                    nc.gpsimd.partition_broadcast(
                                        bc[:], rec[:], P)
                                    nc.gpsimd.tensor_mul(
                                        attnT_u[poff:poff + 64,
                                                s0:s0 + 1024],
                                        attnT_u[poff:poff + 64,
                                                s0:s0 + 1024],
                                        bc[poff:poff + 64, :])
                                    if u == NPAIR - 1 and par == 1:
                                        # last pair: tail D quanta, DVE/ACT
                                        # split, borrowing free score banks
                                        tailq = []
                                        for c in (2 * sh, 2 * sh + 1):
                                            for m in range(8):
                                                ev = "act" if (sh == 1 and
                                                               m % 2) \
                                                    else "dve"
                                                pl = psS if (
                                                    sh == 1 and
                                                    (m // 2) % 2) else None
                                                tailq.append(
                                                    (("D", u, m, c), D_COST,
                                                     (lambda m=m, c=c,
                                                      at_u=attnT_u, ev=ev,
                                                      pl=pl:
                                                      d_quant(NPAIR - 1,
                                                              at_u, m, c,
                                                              ev, pl))))
                                        fillers[:0] = tailq
                                return fin

                            for skp in range(NSKP):
                                pt = noact_pt if NOACT else \
                                    pT_pool.tile([P, 2, 1024], fp8)
                                for sp in range(2):
                                    skt = 2 * skp + sp
                                    need(("A", u, 2 * u + 1, skt // 4, 1))
                                    ps_s = psS.tile([P, 1024], f32,
                                                    name="ps_s", tag="ps_s")
                                    for c in range(2):
                                        # contract-128: packed K (both heads)
                                        # x zero-padded per-head Q slot
                                        nc.tensor.matmul(
                                            ps_s[:, c * 512:(c + 1) * 512],
                                            lhsT=kT2[:, u, skt * P:
                                                     (skt + 1) * P],
                                            rhs=qPAD[:, i,
                                                     s0 + c * 512:
                                                     s0 + (c + 1) * 512],
                                            start=True, stop=True)
                                    if not NOACT:
                                        nc.scalar.activation(
                                            pt[:, sp, :], ps_s[:], AF.Exp,
                                            scale=SCALE)
                                # the previous chunk's PV (possibly from the
                                # previous s-half/head) flushes only after
                                # this chunk's scores+exp feed the ACT engine
                                flush_pending()
                                pending[0] = (
                                    mk_pv(skp, pt),
                                    mk_fin() if skp == NSKP - 1 else None)
                                drain_budget(CHUNK_BUDGET)
                flush_pending()
                while fillers:
                    _run(fillers.pop(0))

        if repeat > 1:
            with tc.For_i(0, repeat, 1):
                for _ in range(bodies):
                    body()
        else:
            body()

    nc.compile()
    return nc


def _get_nc(repeat=1, bodies=1):
    key = (repeat, bodies)
    if key not in _BUILD_CACHE:
        _BUILD_CACHE[key] = build_nc(repeat=repeat, bodies=bodies)
    return _BUILD_CACHE[key]


def shard_inputs(x, w_qkv, b_qkv, w_out, b_out):
    """Host-side sharding: per-core input maps."""
    bf = ml_dtypes.bfloat16
    in_maps = []
    for c in range(NCORES):
        b, g = c // 2, c % 2
        heads = [g * HPC + i for i in range(HPC)]
        # qk columns, pair-interleaved: m-tile 2u = q cols of pair u,
        # m-tile 2u+1 = k cols of pair u; within a tile [headA 64 | headB 64]
        qk_cols, qk_bias = [], []
        for u in range(HPC // 2):
            hA, hB = heads[2 * u], heads[2 * u + 1]
            for off in (0, 64):  # 0: q, 64: k
                for h in (hA, hB):
                    qk_cols.append(w_qkv[:, h * 192 + off:h * 192 + off + 64])
                    qk_bias.append(b_qkv[h * 192 + off:h * 192 + off + 64])
        w_qk_c = np.ascontiguousarray(
            np.concatenate(qk_cols, axis=1)).astype(bf)
        b_qk_c = np.ascontiguousarray(
            np.concatenate(qk_bias)[:, None].astype(np.float32))
        w_v_c = np.ascontiguousarray(np.concatenate(
            [w_qkv[:, h * 192 + 128:h * 192 + 192] for h in heads],
            axis=1)).astype(bf)
        b_v_c = np.ascontiguousarray(np.broadcast_to(np.concatenate(
            [b_qkv[h * 192 + 128:h * 192 + 192] for h in heads])[None, :],
            (P, HPC * D)).astype(np.float32))
        w_out_c = np.ascontiguousarray(np.concatenate(
            [w_out[h * D:(h + 1) * D, :] for h in heads], axis=0)).astype(bf)
        xT_c = np.ascontiguousarray(x[b].T).astype(bf)
        in_maps.append({
            "xT": xT_c, "w_qk": w_qk_c, "b_qk": b_qk_c,
            "w_v": w_v_c, "b_v": b_v_c, "w_out": w_out_c,
        })
    return in_maps


def unshard_output(results, b_out):
    out = np.empty((B, S, E), dtype=np.float32)
    for b in range(B):
        acc = results[2 * b]["outT"].astype(np.float32).sum(axis=0)
        acc += results[2 * b + 1]["outT"].astype(np.float32).sum(axis=0)
        out[b] = acc.T + b_out
    return out


def kernel(x, w_qkv, b_qkv, w_out, b_out):
    global LAST_RESULTS
    x = np.asarray(x, dtype=np.float32)
    w_qkv = np.asarray(w_qkv, dtype=np.float32)
    b_qkv = np.asarray(b_qkv, dtype=np.float32)
    w_out = np.asarray(w_out, dtype=np.float32)
    b_out = np.asarray(b_out, dtype=np.float32)

    nc = _get_nc()
    in_maps = shard_inputs(x, w_qkv, b_qkv, w_out, b_out)
    try:
        res = run_bass_kernel_spmd(nc, in_maps, list(range(NCORES)))
    except ModuleNotFoundError:
        # BASS_TRACE requested but this axon client has no NTFF hook module
        os.environ["BASS_NEVER_TRACE"] = "1"
        res = run_bass_kernel_spmd(nc, in_maps, list(range(NCORES)))
    LAST_RESULTS = res
    return unshard_output(res.results, b_out)
